# revision 1
# baseline (speedup 1.0000x reference)
"""Trainium2 Bass kernel for nn_DependencyLinearLayer.

Math (collapsed-H reformulation of the reference):
  out[b,i,c,j] = dep_logits[dg[b,i,j], c] + s_log[b,i,c] + t_log[b,j,c] + cls_b[c]
where
  dep_logits = dep_emb @ w_d.T                  [48, 12]
  s_log      = x @ (w_s @ s_fc_w).T + w_s@s_fc_b  (combined-weight form)
  t_log      = x @ (w_t @ t_fc_w).T + w_t@t_fc_b
  w_s, w_t, w_d = cls_w[:, :H], cls_w[:, H:2H], cls_w[:, 2H:]

Sharding: 8 cores; core n handles batch b = n//2 and i-rows [128*(n%2), 128*(n%2)+128).

The per-element 48-entry table lookup runs on GPSIMD via ap_gather with a
PAIRED table: table2[48*a+b] = (T[a], T[b]) so each gather index produces two
consecutive output j's, halving the Q7 read-command count (the bottleneck).
All floating-point math (projections, table construction, broadcast adds)
runs on-device (PE/ACT/DVE); the host only reshapes/shards inputs.
"""

import os
import sys

import numpy as np

for _p in ("/opt/trn_rl_repo",):
    if _p not in sys.path:
        sys.path.insert(0, _p)

import concourse.bass as bass  # noqa: E402
import concourse.tile as tile  # noqa: E402
from concourse import bacc, mybir  # noqa: E402
from concourse.tile import ScopedClock, add_dep_helper  # noqa: E402

B, L, IN, H, C, NDEP = 4, 256, 768, 256, 12, 48
NCORES = 8
RPC = L // 2  # i-rows per core (128)
NINST = 8    # ap_gather instructions per core (2 i-rows x 8 groups each)

_MAX_TAIL_WAITS = 1


def _patched_drain_and_barrier(self, tick_clock, wait_clock):
    # The walrus build in this image rejects >1 sync-wait on one CTRL
    # instruction; split the kernel-tail drain waits across nops.
    drain_inst = self.nc.sync.drain()
    wait_clock.add_sem_waits(
        drain_inst.ins, ScopedClock({None: tick_clock.global_clock})
    )
    sync_info = drain_inst.ins.sync_info
    if sync_info is not None and len(sync_info.on_wait) > _MAX_TAIL_WAITS:
        waits = list(sync_info.on_wait)
        sync_info.on_wait = waits[:_MAX_TAIL_WAITS]
        rest = waits[_MAX_TAIL_WAITS:]
        while rest:
            chunk, rest = rest[:_MAX_TAIL_WAITS], rest[_MAX_TAIL_WAITS:]
            nop = self.nc.sync.nop(nofuse=True, hint="tail_drain_split").ins
            nop.sync_info = mybir.SyncInfo(on_wait=chunk, on_update=[])
    self.nc.all_engine_barrier()
    assert self.sems is not None
    popped = self.nc._tile_sem_poison_stack.pop()
    assert popped is self._sem_poison
    self.nc.clear_and_free_semaphores(list(self.sems.allocated().values()))
    self.nc.all_engine_barrier()


tile.TileContext._drain_and_barrier = _patched_drain_and_barrier

_PROGRAM = None

# raw block order (block=2u+h, g) -> i_loc = 16u + 8h + g; _ROWPERM[i_loc] = block*8+g
_ROWPERM = np.zeros(RPC, dtype=np.int64)
for _u in range(NINST):
    for _h in range(2):
        for _g in range(8):
            _ROWPERM[16 * _u + 8 * _h + _g] = (2 * _u + _h) * 8 + _g


def build_program():
    f32 = mybir.dt.float32
    nc = bacc.Bacc("TRN2", target_bir_lowering=False, debug=False)

    xbT = nc.declare_dram_parameter("xbT", [IN, L], f32, isOutput=False)
    xsT = nc.declare_dram_parameter("xsT", [IN, RPC], f32, isOutput=False)
    dgw = nc.declare_dram_parameter("dgw", [128, 128], mybir.dt.int16, isOutput=False)
    sfw = nc.declare_dram_parameter("sfw", [H, IN], f32, isOutput=False)
    tfw = nc.declare_dram_parameter("tfw", [H, IN], f32, isOutput=False)
    # packed small weights: cols [wsT|wtT|wdT|depT|sfb|tfb] = 12+12+12+48+1+1
    wpack = nc.declare_dram_parameter("wpack", [H, 86], f32, isOutput=False)
    clbT = nc.declare_dram_parameter("clbT", [C, 1], f32, isOutput=False)
    repmat = nc.declare_dram_parameter("repmat", [C, 128], f32, isOutput=False)
    out_d = nc.declare_dram_parameter("out", [RPC * 16, L], f32, isOutput=True)

    Copy = mybir.ActivationFunctionType.Copy

    with tile.TileContext(nc) as tc:
        with (
            tc.tile_pool(name="const", bufs=1) as cp,
            tc.tile_pool(name="gpool", bufs=8) as gp,
            tc.tile_pool(name="psum", bufs=1, space="PSUM") as pp,
            tc.tile_pool(name="psum2", bufs=1, space="PSUM") as pp2,
        ):
            # ---- input loads: small gather-critical first (sync queue
            # spreads across DMA engines; scalar queue serializes ~1.3us/desc) ----
            warm_tab = cp.tile([128, 4], f32, tag="warm_tab")
            warm_out = cp.tile([128, 16], f32, tag="warm_out")
            wpk = cp.tile([128, 172], f32, tag="wpk")
            nc.sync.dma_start(
                wpk[:].rearrange("p (h q) -> p h q", h=2),
                wpack[:].rearrange("(h p) q -> p h q", h=2),
            )

            def wp(h0, a, b):
                return wpk[:, h0 * 86 + a: h0 * 86 + b]
            clbT_t = cp.tile([C, 1], f32, tag="clbT_t")
            nc.sync.dma_start(clbT_t[:], clbT[:])
            rep_t = cp.tile([C, 128], f32, tag="rep_t")
            nc.sync.dma_start(rep_t[:], repmat[:])
            dgw_t = cp.tile([128, 128], mybir.dt.int16, tag="dgw_t")
            nc.sync.dma_start(dgw_t[:], dgw[:])
            with tc.high_priority():
                warm_src = wpk[:, 24:28]
                nc.vector.tensor_scalar_mul(warm_tab[:], warm_src, 0.0)
                nc.gpsimd.ap_gather(
                    warm_out[:], warm_tab[:],
                    warm_tab[:, 0:1].bitcast(mybir.dt.int16)[:, 0:1],
                    channels=128, num_elems=4, d=1, num_idxs=16,
                )
            # big loads after the critical batch
            sfw_t = cp.tile([128, 2 * IN], f32, tag="sfw_t")
            tfw_t = cp.tile([128, 2 * IN], f32, tag="tfw_t")
            for h0 in range(2):
                nc.sync.dma_start(sfw_t[:, h0 * IN:(h0 + 1) * IN], sfw[h0 * 128:(h0 + 1) * 128, :])
                nc.sync.dma_start(tfw_t[:, h0 * IN:(h0 + 1) * IN], tfw[h0 * 128:(h0 + 1) * 128, :])
            x_t = cp.tile([128, 6 * L], f32, tag="x_t")
            for m in range(6):
                nc.sync.dma_start(x_t[:, m * L:(m + 1) * L], xbT[m * 128:(m + 1) * 128, :])
            xs_t = cp.tile([128, 6 * RPC], f32, tag="xs_t")
            for m in range(6):
                nc.sync.dma_start(xs_t[:, m * RPC:(m + 1) * RPC], xsT[m * 128:(m + 1) * 128, :])

            # ---- dep_logitsT+biases [12, 48] in ONE accumulation group ----
            # pd[c,d] = sum_h wd[h,c]*depT[h,d] + ws@sfb + wt@tfb (bcast over d)
            with tc.high_priority():
                pd = pp.tile([C, NDEP], f32, tag="pd")
                nc.tensor.matmul(pd[:], wp(0, 24, 36), wp(0, 36, 84),
                                 start=True, stop=False)
                nc.tensor.matmul(pd[:], wp(1, 24, 36), wp(1, 36, 84),
                                 start=False, stop=False)
                nc.tensor.matmul(pd[:], wp(0, 0, 12),
                                 wp(0, 84, 85).broadcast_to([128, NDEP]),
                                 start=False, stop=False)
                nc.tensor.matmul(pd[:], wp(1, 0, 12),
                                 wp(1, 84, 85).broadcast_to([128, NDEP]),
                                 start=False, stop=False)
                nc.tensor.matmul(pd[:], wp(0, 12, 24),
                                 wp(0, 85, 86).broadcast_to([128, NDEP]),
                                 start=False, stop=False)
                pd_last = nc.tensor.matmul(pd[:], wp(1, 12, 24),
                                 wp(1, 85, 86).broadcast_to([128, NDEP]),
                                 start=False, stop=True)

            with tc.high_priority():
                midbufD = cp.tile([C, NDEP], f32, tag="midbufD")
                nc.scalar.activation(midbufD[:], pd[:],
                                     mybir.ActivationFunctionType.Identity, bias=clbT_t[:])
                prepD = pp2.tile([128, NDEP], f32, tag="prepD")
                prep_mm = nc.tensor.matmul(prepD[:], rep_t[:], midbufD[:],
                                           start=True, stop=True)
                t16_t = cp.tile([128, NDEP], f32, tag="t16_t")
                nc.vector.tensor_copy(t16_t[:], prepD[:])

                table2 = cp.tile([128, NDEP * NDEP * 2], f32, tag="table2")
                tv = table2[:].rearrange("p (a b t) -> p a b t", a=NDEP, b=NDEP, t=2)
                nc.vector.tensor_copy(
                    tv[:, :, :, 0], t16_t[:].unsqueeze(2).broadcast_to([128, NDEP, NDEP])
                )
                copy_a = nc.vector.tensor_copy(
                    tv[:, 0:12, :, 1],
                    t16_t[:].unsqueeze(1).broadcast_to([128, 12, NDEP]),
                )
                copy_b = nc.scalar.copy(
                    tv[:, 12:NDEP, :, 1],
                    t16_t[:].unsqueeze(1).broadcast_to([128, NDEP - 12, NDEP]),
                )

            # ---- combined weights W2[k, 0:12]=swT, [12:24]=twT ----
            w2_t = cp.tile([128, 6 * 24], f32, tag="w2_t")
            for m in range(6):
                pw = pp.tile([128, 24], f32, tag="pw")
                for h0 in range(2):
                    mm = nc.tensor.matmul(
                        pw[:, 0:C],
                        sfw_t[:, h0 * IN + m * 128: h0 * IN + (m + 1) * 128],
                        wp(h0, 0, 12),
                        start=(h0 == 0), stop=(h0 == 1),
                    )
                    if m == 0 and h0 == 0:
                        add_dep_helper(mm.ins, pd_last.ins, sync=False,
                                       reason="pd group first on PE")
                        add_dep_helper(mm.ins, prep_mm.ins, sync=False,
                                       reason="prepD before combine on PE")
                for h0 in range(2):
                    nc.tensor.matmul(
                        pw[:, C:2 * C],
                        tfw_t[:, h0 * IN + m * 128: h0 * IN + (m + 1) * 128],
                        wp(h0, 12, 24),
                        start=(h0 == 0), stop=(h0 == 1),
                    )
                w2c = nc.vector.tensor_copy(w2_t[:, m * 24:(m + 1) * 24], pw[:])
                if m == 0:
                    add_dep_helper(w2c.ins, copy_a.ins, sync=False,
                                   reason="table copyA first on DVE")


            # ---- projections: s_logT [12, 256] and t_logT [12, 256] ----
            ps = pp2.tile([C, RPC], f32, tag="ps")
            pt = pp2.tile([C, L], f32, tag="pt")
            for m in range(6):
                nc.tensor.matmul(
                    ps[:], w2_t[:, m * 24: m * 24 + C], xs_t[:, m * RPC:(m + 1) * RPC],
                    start=(m == 0), stop=(m == 5),
                )
            for m in range(6):
                nc.tensor.matmul(
                    pt[:], w2_t[:, m * 24 + C: m * 24 + 2 * C], x_t[:, m * L:(m + 1) * L],
                    start=(m == 0), stop=(m == 5),
                )



            # ---- t/s projections epilogue ----
            midbufT = cp.tile([C, L], f32, tag="midbufT")
            mbt = nc.scalar.activation(midbufT[:], pt[:], Copy)
            add_dep_helper(mbt.ins, copy_b.ins, sync=False,
                           reason="table copyB first on ACT")
            slog_t = cp.tile([C, RPC], f32, tag="slog_t")
            nc.scalar.activation(slog_t[:], ps[:], Copy)
            prepT = pp2.tile([128, L], f32, tag="prepT")
            nc.tensor.matmul(prepT[:], rep_t[:], midbufT[:], start=True, stop=True)
            t16u16 = cp.tile([128, L], f32, tag="t16u16")
            nc.vector.tensor_copy(t16u16[:], prepT[:])

            # ---- S_all [128, 16]: per-(instruction, half) per-partition scalars ----
            # S_all[16g+c, 2u+h] = s_logT[c, 16u+8h+g]
            s_all = cp.tile([128, 16], f32, tag="s_all")
            nc.vector.tensor_scalar_mul(s_all[:], sfw_t[:, 0:16], 0.0)
            for g in range(8):
                nc.scalar.dma_start(
                    s_all[16 * g:16 * g + C, :],
                    slog_t[0:C, g:g + 121:8],
                )

            # ---- gathers + fused adds + stores ----
            u16 = t16u16[:]
            g_tiles = []
            gather_insts = []
            for v in range(3):
                g_t = gp.tile([128, 4 * L], f32, tag="g_t")
                gi = nc.gpsimd.ap_gather(
                    g_t[:].rearrange("p (k t) -> p k t", t=2),
                    table2[:].rearrange("p (e t) -> p e t", t=2),
                    dgw_t[:, 32 * v:32 * (v + 1)],
                    channels=128, num_elems=NDEP * NDEP, d=2, num_idxs=512,
                )
                g_tiles.append(g_t)
                gather_insts.append(gi)
            # last block-pair split in two so its first half's epilogue
            # overlaps the second half's gather
            g_t3 = gp.tile([128, 4 * L], f32, tag="g_t3")
            for half in range(2):
                gi = nc.gpsimd.ap_gather(
                    g_t3[:, half * 2 * L:(half + 1) * 2 * L].rearrange(
                        "p (k t) -> p k t", t=2),
                    table2[:].rearrange("p (e t) -> p e t", t=2),
                    dgw_t[:, 96 + 16 * half:96 + 16 * (half + 1)],
                    channels=128, num_elems=NDEP * NDEP, d=2, num_idxs=256,
                )
                gather_insts.append(gi)
            g_tiles.append(g_t3)
            # adds + stores AFTER all gathers (DVE shares the GPSIMD SBUF port)
            last_g = gather_insts[3]  # first half of the split pair
            for v in range(4):
                g_t = g_tiles[v]
                for bidx in range(4):
                    aa = nc.vector.affine_then_add(
                        g_t[:, bidx * L:(bidx + 1) * L],
                        g_t[:, bidx * L:(bidx + 1) * L],
                        u16,
                        1.0,
                        s_all[:, 4 * v + bidx:4 * v + bidx + 1],
                    )
                    if v < 3:
                        add_dep_helper(aa.ins, last_g.ins, sync=False,
                                       reason="defer adds past gathers")
                if v < 3:
                    nc.sync.dma_start(
                        out_d[4 * v * 128:(4 * v + 4) * 128, :].rearrange(
                            "(b p) j -> p b j", b=4
                        ),
                        g_t[:].rearrange("p (b j) -> p b j", b=4),
                    )
                else:
                    for half in range(2):
                        nc.sync.dma_start(
                            out_d[(12 + 2 * half) * 128:(14 + 2 * half) * 128, :]
                            .rearrange("(b p) j -> p b j", b=2),
                            g_t[:, half * 2 * L:(half + 1) * 2 * L]
                            .rearrange("p (b j) -> p b j", b=2),
                        )

    nc.compile()
    return nc


def _marshal_core(n, input_tensor, dg, consts):
    b, half = n // 2, n % 2
    i0 = half * RPC
    dgb = dg[b]
    # paired indices, wrapped per 16-partition group:
    # instruction u, group g -> stream of 256: 128 pairs of row 16u+g,
    # then 128 pairs of row 16u+8+g; stream[k] at [16g + k%16, 16u + k//16].
    pairs = (dgb[:, 0::2] * NDEP + dgb[:, 1::2]).astype(np.int16)  # [L, 128]
    dgw = np.empty((128, 128), dtype=np.int16)
    for v in range(4):
        for g in range(8):
            stream = np.concatenate([
                pairs[i0 + 16 * ((4 * v + b) // 2) + 8 * ((4 * v + b) % 2) + g]
                for b in range(4)
            ])  # [512]
            dgw[16 * g:16 * (g + 1), 32 * v:32 * (v + 1)] = stream.reshape(32, 16).T
    m = {
        "xbT": np.ascontiguousarray(input_tensor[b].T),
        "xsT": np.ascontiguousarray(input_tensor[b, i0:i0 + RPC].T),
        "dgw": dgw,
    }
    m.update(consts)
    return m


def kernel(input_tensor, dependency_graph, s_fc_w, s_fc_b, t_fc_w, t_fc_b,
           dep_emb, cls_w, cls_b):
    global _PROGRAM
    from concourse.bass_utils import run_bass_kernel_spmd

    input_tensor = np.asarray(input_tensor, dtype=np.float32)
    dg = np.asarray(dependency_graph)
    out_dtype = np.float32

    cw = np.asarray(cls_w, np.float32)
    wpack = np.concatenate([
        cw[:, 0:H].T, cw[:, H:2 * H].T, cw[:, 2 * H:].T,
        np.asarray(dep_emb, np.float32).T,
        np.asarray(s_fc_b, np.float32).reshape(H, 1),
        np.asarray(t_fc_b, np.float32).reshape(H, 1),
    ], axis=1)
    consts = {
        "sfw": np.ascontiguousarray(np.asarray(s_fc_w, np.float32)),
        "tfw": np.ascontiguousarray(np.asarray(t_fc_w, np.float32)),
        "wpack": np.ascontiguousarray(wpack),
        "clbT": np.asarray(cls_b, np.float32).reshape(C, 1).copy(),
    }
    # repmat[c, 16g+c'] = (c' == c)
    rm = np.zeros((C, 128), dtype=np.float32)
    for g in range(8):
        rm[np.arange(C), 16 * g + np.arange(C)] = 1.0
    consts["repmat"] = rm

    if _PROGRAM is None:
        _PROGRAM = build_program()
    nc = _PROGRAM

    in_maps = [_marshal_core(n, input_tensor, dg, consts) for n in range(NCORES)]
    trace = bool(int(os.environ.get("KERNEL_PROFILE", "0")))
    res = run_bass_kernel_spmd(
        nc, in_maps, core_ids=list(range(NCORES)), trace=trace
    )
    if trace and res.exec_time_ns is not None:
        print(f"HW exec time: {res.exec_time_ns} ns")

    out = np.empty((B, L, C, L), dtype=out_dtype)
    for n in range(NCORES):
        b, half = n // 2, n % 2
        i0 = half * RPC
        # raw flat row = (2u+h)*128 + 16g + c ; i_loc = 16u + 8h + g
        raw = res.results[n]["out"].reshape(2 * NINST, 8, 16, L)  # [block=2u+h, g, c16, j]
        out[b, i0:i0 + RPC] = raw[:, :, :C, :].reshape(2 * NINST * 8, C, L)[_ROWPERM]
    return out



# revision 17
# speedup vs baseline: 1.7661x; 1.7661x over previous
"""Trainium2 Bass kernel for nn_DependencyLinearLayer.

Math (collapsed-H reformulation of the reference):
  out[b,i,c,j] = DL[dg[b,i,j], c] + s_log[b,i,c] + t_log[b,j,c] + bias[c]
where
  DL        = dep_emb @ w_d.T                     [48, 12]
  s_log     = x @ (w_s @ s_fc_w).T                (combined-weight form)
  t_log     = x @ (w_t @ t_fc_w).T
  bias      = cls_b + w_s@s_fc_b + w_t@t_fc_b     (folded into s_log)
  w_s, w_t, w_d = cls_w[:, :H], cls_w[:, H:2H], cls_w[:, 2H:]

The per-element 48-entry lookup DL[dg] runs on the TENSOR engine as a
one-hot matmul: the host marshals dg into a one-hot fp8 matrix (pure
index->bit-pattern encoding, no float math) and the PE contracts it with
a block-diagonal fp16 [96, 24] stationary diag(DL, DL), gathering two
i-rows per pass at 1 column/cycle.  All floating-point math (projections,
DL, broadcast adds) runs on-device (PE/ACT/DVE); the host only
reshapes/shards inputs.

Sharding: 8 cores; core n handles batch b = n//2 and i-rows
[128*(n%2), 128*(n%2)+128).  The j axis is rotated by i0 per core so the
s-projection reads a fixed column slice of the shared x^T tile.
"""

import os
import sys

import numpy as np

for _p in ("/opt/trn_rl_repo",):
    if _p not in sys.path:
        sys.path.insert(0, _p)

import concourse.bass as bass  # noqa: E402
import concourse.tile as tile  # noqa: E402
from concourse import bacc, mybir  # noqa: E402
from concourse.tile import ScopedClock, add_dep_helper  # noqa: E402

B, L, IN, H, C, NDEP = 4, 256, 768, 256, 12, 48
NCORES = 8
RPC = L // 2      # i-rows per core (128)
NBANK = 8         # PSUM gather banks; each holds 4 slots x 2 col-halves
NM = 32           # gather matmuls per core (one per [24, 512] stripe)
FP8_ONE = 0x38    # fp8 e4m3 bit pattern for 1.0

_MAX_TAIL_WAITS = 1


def _patched_drain_and_barrier(self, tick_clock, wait_clock):
    # The walrus build in this image rejects >1 sync-wait on one CTRL
    # instruction; split the kernel-tail drain waits across nops.
    drain_inst = self.nc.sync.drain()
    wait_clock.add_sem_waits(
        drain_inst.ins, ScopedClock({None: tick_clock.global_clock})
    )
    sync_info = drain_inst.ins.sync_info
    if sync_info is not None and len(sync_info.on_wait) > _MAX_TAIL_WAITS:
        waits = list(sync_info.on_wait)
        sync_info.on_wait = waits[:_MAX_TAIL_WAITS]
        rest = waits[_MAX_TAIL_WAITS:]
        while rest:
            chunk, rest = rest[:_MAX_TAIL_WAITS], rest[_MAX_TAIL_WAITS:]
            nop = self.nc.sync.nop(nofuse=True, hint="tail_drain_split").ins
            nop.sync_info = mybir.SyncInfo(on_wait=chunk, on_update=[])
    self.nc.all_engine_barrier()
    assert self.sems is not None
    popped = self.nc._tile_sem_poison_stack.pop()
    assert popped is self._sem_poison
    self.nc.clear_and_free_semaphores(list(self.sems.allocated().values()))
    self.nc.all_engine_barrier()


tile.TileContext._drain_and_barrier = _patched_drain_and_barrier

_PROGRAM = None

# wpack column layout (per h-half of the H dim):
# 0:12 wsT | 12:24 wtT | 24:36 wdT | 36:84 depT | 84 sfb | 85 tfb |
# 86:98 cls_b row (partition 0 of half 0 only) | 98 one (p0/h0) |
# 99:227 rep128 (partitions 0:12 of half 0 only)
WCOLS = 227


def build_program():
    f16 = mybir.dt.float16
    f32 = mybir.dt.float32
    nc = bacc.Bacc("TRN2", target_bir_lowering=False, debug=False)

    wpk_d = nc.declare_dram_parameter("wpk", [H, WCOLS], f16, isOutput=False)
    # big = [sfw h0|h1 (1536) | tfw h0|h1 (1536) | xbT m0..m5 (1536)]
    big_d = nc.declare_dram_parameter("big", [128, 4608], f16, isOutput=False)
    # one-hot rows: 64*i + d (rows 48:64 and 112:128 are zero padding so
    # the block-diagonal stationary's copies land on 32-aligned partitions)
    oh_d = nc.declare_dram_parameter("oh", [112, 64 * L], mybir.dt.uint8,
                                     isOutput=False)
    # row = 128g + 32s + 12i + c (rows 24:32 of each 32-block are pad),
    # col = 256h + j'
    out_d = nc.declare_dram_parameter("out", [1024, 2 * L], f16, isOutput=True)

    Identity = mybir.ActivationFunctionType.Identity
    Copy = mybir.ActivationFunctionType.Copy

    with tile.TileContext(nc) as tc:
        with (
            tc.tile_pool(name="const", bufs=1) as cp,
            tc.tile_pool(name="opool", bufs=3) as op,
            tc.tile_pool(name="smallp", bufs=1, space="PSUM") as sp,
            tc.tile_pool(name="projp", bufs=1, space="PSUM") as jp,
            tc.tile_pool(name="prept", bufs=1, space="PSUM") as qp,
            tc.tile_pool(name="gp", bufs=5, space="PSUM") as gp,
        ):
            # ---- loads ----
            wpk = cp.tile([128, 2 * WCOLS], f16, tag="wpk")
            with tc.high_priority():
                nc.sync.dma_start(
                    wpk[:].rearrange("p (h q) -> p h q", h=2),
                    wpk_d[:].rearrange("(h p) q -> p h q", h=2),
                )

            def wp(h0, a, b):
                return wpk[:, h0 * WCOLS + a: h0 * WCOLS + b]

            big = cp.tile([128, 4608], f16, tag="big")
            for k in range(3):
                nc.sync.dma_start(big[:, 1536 * k:1536 * (k + 1)],
                                  big_d[:, 1536 * k:1536 * (k + 1)])

            def xch(m, w):
                return big[:, 3072 + 256 * m: 3072 + 256 * m + w]

            oh_t = cp.tile([112, 64 * L], mybir.dt.uint8, tag="oh_t")
            for k in range(4):
                nc.gpsimd.dma_start(oh_t[:, 4096 * k:4096 * (k + 1)],
                                    oh_d[:, 4096 * k:4096 * (k + 1)])
            oh8 = oh_t[:].bitcast(mybir.dt.float8e4)

            w_dl = cp.tile([128, 32], f16, tag="w_dl")
            nc.gpsimd.memset(w_dl[:], 0)

            # ---- bias row: cls_b + ws@sfb + wt@tfb  [12, 1] ----
            with tc.high_priority():
                pbias = sp.tile([C, 1], f32, tag="small")
                nc.tensor.matmul(pbias[:], wp(0, 0, 12), wp(0, 84, 85),
                                 start=True, stop=False)
                nc.tensor.matmul(pbias[:], wp(1, 0, 12), wp(1, 84, 85),
                                 start=False, stop=False)
                nc.tensor.matmul(pbias[:], wp(0, 12, 24), wp(0, 85, 86),
                                 start=False, stop=False)
                nc.tensor.matmul(pbias[:], wp(1, 12, 24), wp(1, 85, 86),
                                 start=False, stop=False)
                nc.tensor.matmul(pbias[:], wp(0, 86, 98)[0:1, :],
                                 wp(0, 98, 99)[0:1, :],
                                 start=False, stop=True)
                bias_sb = cp.tile([C, 1], f32, tag="bias_sb")
                nc.scalar.copy(bias_sb[:], pbias[:])

                # ---- DL [48, 12] then block-diagonal [96, 24] fp16 ----
                pd2 = sp.tile([NDEP, C], f32, tag="small")
                nc.tensor.matmul(pd2[:], wp(0, 36, 84), wp(0, 24, 36),
                                 start=True, stop=False)
                pd2_mm = nc.tensor.matmul(pd2[:], wp(1, 36, 84), wp(1, 24, 36),
                                          start=False, stop=True)
                nc.vector.tensor_copy(w_dl[0:48, 0:12], pd2[:])
                nc.vector.tensor_copy(w_dl[64:112, 12:24], pd2[:])

            # ---- gather matmuls, first 4 banks ----
            g_tiles = [None] * NBANK

            def gather_bank(g):
                gt = gp.tile([128, 512], f32, tag="gt")
                g_tiles[g] = gt
                for s in range(4):
                    m = 4 * g + s
                    nc.tensor.matmul(
                        gt[32 * s:32 * s + 32, :], w_dl[0:112, :],
                        oh8[:, 512 * m:512 * (m + 1)],
                        start=True, stop=True,
                        tile_position=(0, 32 * s),
                    )

            for g in range(4):
                gather_bank(g)

            # ---- combined weights w2 = [w2sT | w2tT] per IN-chunk ----
            w2_t = cp.tile([128, 6 * 2 * C], f16, tag="w2_t")
            for m in range(6):
                pw = sp.tile([128, 2 * C], f32, tag="small")
                for h0 in range(2):
                    nc.tensor.matmul(
                        pw[:, 0:C],
                        big[:, 768 * h0 + 128 * m: 768 * h0 + 128 * (m + 1)],
                        wp(h0, 0, 12),
                        start=(h0 == 0), stop=(h0 == 1),
                    )
                for h0 in range(2):
                    nc.tensor.matmul(
                        pw[:, C:2 * C],
                        big[:, 1536 + 768 * h0 + 128 * m:
                             1536 + 768 * h0 + 128 * (m + 1)],
                        wp(h0, 12, 24),
                        start=(h0 == 0), stop=(h0 == 1),
                    )
                nc.vector.tensor_copy(w2_t[:, 24 * m:24 * (m + 1)], pw[:])

            # ---- projections: t_log [12, 256], s_log [12, 128] ----
            pt = jp.tile([C, L], f32, tag="proj")
            for m in range(6):
                nc.tensor.matmul(
                    pt[:], w2_t[:, 24 * m + 12:24 * m + 24], xch(m, 256),
                    start=(m == 0), stop=(m == 5),
                )
            midbufT = cp.tile([C, L], f16, tag="midbufT")
            nc.scalar.copy(midbufT[:], pt[:])

            ps = jp.tile([C, RPC], f32, tag="proj")
            for m in range(6):
                nc.tensor.matmul(
                    ps[:], w2_t[:, 24 * m:24 * m + 12], xch(m, 128),
                    start=(m == 0), stop=(m == 5),
                )
            slog_t = cp.tile([C, RPC], f32, tag="slog_t")
            nc.scalar.activation(slog_t[:], ps[:], Identity, bias=bias_sb[:])

            # ---- t120 [128, 256]: t_log replicated to (s, i, c) rows ----
            prepT = qp.tile([128, L], f32, tag="prepT")
            nc.tensor.matmul(prepT[:], wp(0, 99, 227)[0:12, :], midbufT[:],
                             start=True, stop=True)
            t120 = cp.tile([128, L], f32, tag="t120")
            nc.vector.tensor_copy(t120[:], prepT[:])

            # ---- s_all [128, 16]: s_all[32s+12i+c, 2g+h] =
            #      slog_t[c, 16g+4s+2h+i], one strided DMA ----
            # i_loc(g,s,h,i) = 32s + 16i + 2g + h, so per-(s,i) the 16
            # (g,h) scalars are a contiguous slog_t slice.
            # s_all[32s+12i+c, 2g+h] = slog_t[c, 32s+16i+2g+h]
            s_all = cp.tile([128, 16], f32, tag="s_all")
            nc.gpsimd.memset(s_all[:], 0)
            for s in range(4):
                for i in range(2):
                    nc.sync.dma_start(
                        s_all[32 * s + 12 * i:32 * s + 12 * i + 12, :],
                        slog_t[0:12, 32 * s + 16 * i:32 * s + 16 * i + 16],
                    )

            # ---- remaining gather banks ----
            for g in range(4, NBANK):
                gather_bank(g)

            # ---- epilogue: out = gt + t120 + s  (per bank, per col-half) ----
            for g in range(NBANK):
                gt = g_tiles[g]
                ot = op.tile([128, 512], f16, tag="ot")
                for h in range(2):
                    nc.vector.affine_then_add(
                        ot[:, 256 * h:256 * (h + 1)],
                        gt[:, 256 * h:256 * (h + 1)],
                        t120[:],
                        1.0,
                        s_all[:, 2 * g + h:2 * g + h + 1],
                    )
                eng = nc.gpsimd if g < 4 else nc.sync
                eng.dma_start(out_d[128 * g:128 * (g + 1), :], ot[:])

    nc.compile()
    return nc


def _make_consts(s_fc_w, s_fc_b, t_fc_w, t_fc_b, dep_emb, cls_w, cls_b):
    cw = np.asarray(cls_w, np.float32)
    wpack = np.zeros((H, WCOLS), np.float32)
    wpack[:, 0:12] = cw[:, 0:H].T
    wpack[:, 12:24] = cw[:, H:2 * H].T
    wpack[:, 24:36] = cw[:, 2 * H:].T
    wpack[:, 36:84] = np.asarray(dep_emb, np.float32).T
    wpack[:, 84] = np.asarray(s_fc_b, np.float32)
    wpack[:, 85] = np.asarray(t_fc_b, np.float32)
    wpack[0, 86:98] = np.asarray(cls_b, np.float32)
    wpack[0, 98] = 1.0
    # rep128[c, 32s+12i+c] = 1
    for s in range(4):
        for i in range(2):
            for c in range(C):
                wpack[c, 99 + 32 * s + 12 * i + c] = 1.0
    return {"wpk": wpack.astype(np.float16)}


_COLBASE = None


def _marshal_core(n, input_tensor, dg, consts):
    global _COLBASE
    b, half = n // 2, n % 2
    i0 = half * RPC

    xbT = np.roll(input_tensor[b].T, -i0, axis=1)  # [768, 256] rotated j
    sfw = consts["_sfw"]
    tfw = consts["_tfw"]
    big = np.concatenate(
        [sfw[0:128], sfw[128:256], tfw[0:128], tfw[128:256]]
        + [xbT[128 * m:128 * (m + 1)] for m in range(6)],
        axis=1,
    ).astype(np.float16)  # [128, 4608]

    dgc = np.roll(np.asarray(dg[b, i0:i0 + RPC]), -i0, axis=1)  # [128, 256]
    if _COLBASE is None:
        # i_loc r = 32s + 16i + 2g + h -> matmul m = 4g + s,
        # oh col = 512m + 256h + j', oh row block = 48i
        r = np.arange(RPC)
        s_, i_, g_, h_ = r // 32, (r % 32) // 16, (r % 16) // 2, r % 2
        _COLBASE = (
            (512 * (4 * g_ + s_) + 256 * h_)[:, None] + np.arange(L)[None, :],
            (64 * i_)[:, None],
        )
    rows = _COLBASE[1] + dgc
    oh = np.zeros((112, 64 * L), np.uint8)
    oh[rows.ravel(), _COLBASE[0].ravel()] = FP8_ONE

    return {"wpk": consts["wpk"], "big": big, "oh": oh}


def _assemble_core(raw, i0):
    # raw [1024, 512] f16: row = 128g + 32s + 12i + c (24:32 pad),
    # col = 256h + j'; i_loc = 32s + 16i + 2g + h,
    # col j' holds global j = (j'+i0) % L
    arr = raw.reshape(8, 4, 32, 2, L)[:, :, :24]    # (g, s, ic, h, j)
    arr = arr.reshape(8, 4, 2, C, 2, L).transpose(1, 2, 0, 4, 3, 5)
    arr = arr.reshape(RPC, C, L)
    return np.roll(arr, i0, axis=2).astype(np.float32)


def kernel(input_tensor, dependency_graph, s_fc_w, s_fc_b, t_fc_w, t_fc_b,
           dep_emb, cls_w, cls_b):
    global _PROGRAM
    from concourse.bass_utils import run_bass_kernel_spmd

    input_tensor = np.asarray(input_tensor, dtype=np.float32)
    dg = np.asarray(dependency_graph)

    consts = _make_consts(s_fc_w, s_fc_b, t_fc_w, t_fc_b, dep_emb,
                          cls_w, cls_b)
    consts["_sfw"] = np.asarray(s_fc_w, np.float32)
    consts["_tfw"] = np.asarray(t_fc_w, np.float32)

    if _PROGRAM is None:
        _PROGRAM = build_program()
    nc = _PROGRAM

    in_maps = [_marshal_core(n, input_tensor, dg, consts) for n in range(NCORES)]
    trace = bool(int(os.environ.get("KERNEL_PROFILE", "0")))
    res = run_bass_kernel_spmd(
        nc, in_maps, core_ids=list(range(NCORES)), trace=trace
    )
    if trace and res.exec_time_ns is not None:
        print(f"HW exec time: {res.exec_time_ns} ns")

    out = np.empty((B, L, C, L), dtype=np.float32)
    for n in range(NCORES):
        b, half = n // 2, n % 2
        i0 = half * RPC
        out[b, i0:i0 + RPC] = _assemble_core(res.results[n]["out"], i0)
    return out


# revision 24
# speedup vs baseline: 2.0288x; 1.1487x over previous
"""Trainium2 Bass kernel for nn_DependencyLinearLayer.

Math (collapsed-H reformulation of the reference):
  out[b,i,c,j] = DL[dg[b,i,j], c] + s_log[b,i,c] + t_log[b,j,c] + bias[c]
where
  DL        = dep_emb @ w_d.T                     [48, 12]
  s_log     = x @ (w_s @ s_fc_w).T                (combined-weight form)
  t_log     = x @ (w_t @ t_fc_w).T
  bias      = cls_b + w_s@s_fc_b + w_t@t_fc_b     (folded into s_log)
  w_s, w_t, w_d = cls_w[:, :H], cls_w[:, H:2H], cls_w[:, 2H:]

The per-element 48-entry lookup DL[dg] runs on the TENSOR engine as a
one-hot matmul: the host marshals dg into a one-hot fp8 matrix (pure
index->bit-pattern encoding, no float math) and the PE contracts it with
a block-diagonal fp16 [96, 24] stationary diag(DL, DL), gathering two
i-rows per pass at 1 column/cycle.  All floating-point math (projections,
DL, broadcast adds) runs on-device (PE/ACT/DVE); the host only
reshapes/shards inputs.

Sharding: 8 cores; core n handles batch b = n//2 and i-rows
[128*(n%2), 128*(n%2)+128).  The j axis is rotated by i0 per core so the
s-projection reads a fixed column slice of the shared x^T tile.
"""

import os
import sys

import numpy as np

for _p in ("/opt/trn_rl_repo",):
    if _p not in sys.path:
        sys.path.insert(0, _p)

import concourse.bass as bass  # noqa: E402
import concourse.tile as tile  # noqa: E402
from concourse import bacc, mybir  # noqa: E402
from concourse.tile import ScopedClock, add_dep_helper  # noqa: E402

B, L, IN, H, C, NDEP = 4, 256, 768, 256, 12, 48
NCORES = 8
RPC = L // 2      # i-rows per core (128)
NBANK = 8         # PSUM gather banks; each holds 4 slots x 2 col-halves
NM = 32           # gather matmuls per core (one per [24, 512] stripe)
FP8_ONE = 0x38    # fp8 e4m3 bit pattern for 1.0

_MAX_TAIL_WAITS = 1


def _patched_drain_and_barrier(self, tick_clock, wait_clock):
    # The walrus build in this image rejects >1 sync-wait on one CTRL
    # instruction; split the kernel-tail drain waits across nops.
    drain_inst = self.nc.sync.drain()
    wait_clock.add_sem_waits(
        drain_inst.ins, ScopedClock({None: tick_clock.global_clock})
    )
    sync_info = drain_inst.ins.sync_info
    if sync_info is not None and len(sync_info.on_wait) > _MAX_TAIL_WAITS:
        waits = list(sync_info.on_wait)
        sync_info.on_wait = waits[:_MAX_TAIL_WAITS]
        rest = waits[_MAX_TAIL_WAITS:]
        while rest:
            chunk, rest = rest[:_MAX_TAIL_WAITS], rest[_MAX_TAIL_WAITS:]
            nop = self.nc.sync.nop(nofuse=True, hint="tail_drain_split").ins
            nop.sync_info = mybir.SyncInfo(on_wait=chunk, on_update=[])
    self.nc.all_engine_barrier()
    assert self.sems is not None
    popped = self.nc._tile_sem_poison_stack.pop()
    assert popped is self._sem_poison
    self.nc.clear_and_free_semaphores(list(self.sems.allocated().values()))
    self.nc.all_engine_barrier()


tile.TileContext._drain_and_barrier = _patched_drain_and_barrier

_PROGRAM = None

# wpack column layout (per h-half of the H dim):
# 0:12 wsT | 12:24 wtT | 24:36 wdT | 36:84 depT | 84 sfb | 85 tfb |
# 86:98 cls_b row (partition 0 of half 0 only) | 98 one (p0/h0) |
# 99:227 rep128 (partitions 0:12 of half 0 only)
WCOLS = 227


def build_program():
    f16 = mybir.dt.float16
    f32 = mybir.dt.float32
    nc = bacc.Bacc("TRN2", target_bir_lowering=False, debug=False)

    wpk_d = nc.declare_dram_parameter("wpk", [H, WCOLS], f16, isOutput=False)
    # big = [sfw h0|h1 (1536) | tfw h0|h1 (1536) | xbT m0..m5 (1536)]
    big_d = nc.declare_dram_parameter("big", [128, 4608], f16, isOutput=False)
    # one-hot rows: 64*i + d (rows 48:64 and 112:128 are zero padding so
    # the block-diagonal stationary's copies land on 32-aligned partitions)
    oh_d = nc.declare_dram_parameter("oh", [112, 64 * L], mybir.dt.uint8,
                                     isOutput=False)
    # row = 128g + 32s + 12i + c (rows 24:32 of each 32-block are pad),
    # col = 256h + j'
    out_d = nc.declare_dram_parameter("out", [1024, 2 * L], f16, isOutput=True)

    Identity = mybir.ActivationFunctionType.Identity
    Copy = mybir.ActivationFunctionType.Copy

    with tile.TileContext(nc) as tc:
        with (
            tc.tile_pool(name="const", bufs=1) as cp,
            tc.tile_pool(name="opool", bufs=3) as op,
            tc.tile_pool(name="smallp", bufs=1, space="PSUM") as sp,
            tc.tile_pool(name="projp", bufs=1, space="PSUM") as jp,
            tc.tile_pool(name="prept", bufs=1, space="PSUM") as qp,
            tc.tile_pool(name="gp", bufs=5, space="PSUM") as gp,
        ):
            # ---- loads ----
            wpk = cp.tile([128, 2 * WCOLS], f16, tag="wpk")
            with tc.high_priority():
                nc.sync.dma_start(
                    wpk[:].rearrange("p (h q) -> p h q", h=2),
                    wpk_d[:].rearrange("(h p) q -> p h q", h=2),
                )

            def wp(h0, a, b):
                return wpk[:, h0 * WCOLS + a: h0 * WCOLS + b]

            big = cp.tile([128, 4608], f16, tag="big")
            for k in range(3):
                nc.sync.dma_start(big[:, 1536 * k:1536 * (k + 1)],
                                  big_d[:, 1536 * k:1536 * (k + 1)])

            def xch(m, w):
                return big[:, 3072 + 256 * m: 3072 + 256 * m + w]

            oh_t = cp.tile([112, 64 * L], mybir.dt.uint8, tag="oh_t")
            for k in range(4):
                nc.gpsimd.dma_start(oh_t[:, 4096 * k:4096 * (k + 1)],
                                    oh_d[:, 4096 * k:4096 * (k + 1)])
            oh8 = oh_t[:].bitcast(mybir.dt.float8e4)

            w_dl = cp.tile([128, 32], f16, tag="w_dl")
            s_all = cp.tile([128, 16], f32, tag="s_all")
            nc.gpsimd.memset(w_dl[:], 0)
            nc.gpsimd.memset(s_all[:], 0)

            # ---- bias row: cls_b + ws@sfb + wt@tfb  [12, 1] ----
            with tc.high_priority():
                pbias = sp.tile([C, 1], f32, tag="small")
                nc.tensor.matmul(pbias[:], wp(0, 0, 12), wp(0, 84, 85),
                                 start=True, stop=False)
                nc.tensor.matmul(pbias[:], wp(1, 0, 12), wp(1, 84, 85),
                                 start=False, stop=False)
                nc.tensor.matmul(pbias[:], wp(0, 12, 24), wp(0, 85, 86),
                                 start=False, stop=False)
                nc.tensor.matmul(pbias[:], wp(1, 12, 24), wp(1, 85, 86),
                                 start=False, stop=False)
                nc.tensor.matmul(pbias[:], wp(0, 86, 98)[0:1, :],
                                 wp(0, 98, 99)[0:1, :],
                                 start=False, stop=True)
                bias_sb = cp.tile([C, 1], f32, tag="bias_sb")
                nc.scalar.copy(bias_sb[:], pbias[:])

                # ---- DL [48, 12] then block-diagonal [96, 24] fp16 ----
                pd2 = sp.tile([NDEP, C], f32, tag="small")
                nc.tensor.matmul(pd2[:], wp(0, 36, 84), wp(0, 24, 36),
                                 start=True, stop=False)
                pd2_mm = nc.tensor.matmul(pd2[:], wp(1, 36, 84), wp(1, 24, 36),
                                          start=False, stop=True)
                nc.vector.tensor_copy(w_dl[0:48, 0:12], pd2[:])
                nc.vector.tensor_copy(w_dl[64:112, 12:24], pd2[:])

            g_tiles = [None] * NBANK

            def gather_bank(g):
                # odd banks: PSUM pre-seeded with the t_log broadcast (their
                # eviction runs on ACT, which can only add a scalar bias)
                gt = gp.tile([128, 512], f32, tag="gt")
                g_tiles[g] = gt
                seed = g % 2 == 1
                if seed:
                    nc.tensor.matmul(
                        gt[:], wp(0, 99, 227)[0:12, :], midbufT[:],
                        start=True, stop=False, skip_group_check=True,
                    )
                for s in range(4):
                    m = 4 * g + s
                    nc.tensor.matmul(
                        gt[32 * s:32 * s + 32, :], w_dl[0:112, :],
                        oh8[:, 512 * m:512 * (m + 1)],
                        start=not seed, stop=(not seed) or (s == 3),
                        skip_group_check=seed,
                        tile_position=(0, 32 * s),
                    )

            # ---- combined weights w2 = [w2sT | w2tT] per IN-chunk ----
            # all 6 chunks accumulate into one PSUM tile; single cast out
            w2_t = cp.tile([128, 6 * 2 * C], f16, tag="w2_t")
            pw = sp.tile([128, 6 * 2 * C], f32, tag="small")
            for m in range(6):
                for h0 in range(2):
                    nc.tensor.matmul(
                        pw[:, 24 * m:24 * m + C],
                        big[:, 768 * h0 + 128 * m: 768 * h0 + 128 * (m + 1)],
                        wp(h0, 0, 12),
                        start=(h0 == 0), stop=(h0 == 1),
                    )
                for h0 in range(2):
                    nc.tensor.matmul(
                        pw[:, 24 * m + C:24 * m + 2 * C],
                        big[:, 1536 + 768 * h0 + 128 * m:
                             1536 + 768 * h0 + 128 * (m + 1)],
                        wp(h0, 12, 24),
                        start=(h0 == 0), stop=(h0 == 1),
                    )
            nc.vector.tensor_copy(w2_t[:], pw[:])

            # ---- projections: s_log [12, 128] first (s_all gates the
            # epilogue), then t_log [12, 256] ----
            ps = jp.tile([C, RPC], f32, tag="proj")
            for m in range(6):
                nc.tensor.matmul(
                    ps[:], w2_t[:, 24 * m:24 * m + 12], xch(m, 128),
                    start=(m == 0), stop=(m == 5),
                )
            slog_t = cp.tile([C, RPC], f32, tag="slog_t")
            nc.scalar.activation(slog_t[:], ps[:], Identity, bias=bias_sb[:])

            pt = jp.tile([C, L], f32, tag="proj")
            for m in range(6):
                nc.tensor.matmul(
                    pt[:], w2_t[:, 24 * m + 12:24 * m + 24], xch(m, 256),
                    start=(m == 0), stop=(m == 5),
                )
            midbufT = cp.tile([C, 2 * L], f16, tag="midbufT")
            nc.scalar.copy(midbufT[:, 0:L], pt[:])
            nc.scalar.copy(midbufT[:, L:2 * L], pt[:])

            # ---- t120 [128, 256]: t_log replicated to (s, i, c) rows ----
            prepT = qp.tile([128, L], f32, tag="prepT")
            nc.tensor.matmul(prepT[:], wp(0, 99, 227)[0:12, :],
                             midbufT[:, 0:L], start=True, stop=True)
            t120 = cp.tile([128, L], f32, tag="t120")
            nc.vector.tensor_copy(t120[:], prepT[:])

            # ---- s_all [128, 16]: s_all[32s+12i+c, 2g+h] =
            #      slog_t[c, 16g+4s+2h+i], one strided DMA ----
            # i_loc(g,s,h,i) = 32s + 16i + 2g + h, so per-(s,i) the 16
            # (g,h) scalars are a contiguous slog_t slice.
            # s_all[32s+12i+c, 2g+h] = slog_t[c, 32s+16i+2g+h]
            for s in range(4):
                for i in range(2):
                    eng = nc.sync if s < 2 else nc.scalar
                    eng.dma_start(
                        s_all[32 * s + 12 * i:32 * s + 12 * i + 12, :],
                        slog_t[0:12, 32 * s + 16 * i:32 * s + 16 * i + 16],
                    )

            # ---- gather banks + epilogue (out = gt + t120 + s) ----
            Add = mybir.AluOpType.add
            for g in range(NBANK):
                gather_bank(g)
            for g in range(NBANK):
                gt = g_tiles[g]
                ot = op.tile([128, 512], f16, tag="ot")
                for h in range(2):
                    dst = ot[:, 256 * h:256 * (h + 1)]
                    srcp = gt[:, 256 * h:256 * (h + 1)]
                    scol = s_all[:, 2 * g + h:2 * g + h + 1]
                    if g % 2 == 0:
                        nc.vector.affine_then_add(dst, srcp, t120[:], 1.0,
                                                  scol)
                    else:
                        nc.scalar.activation(dst, srcp, Identity, bias=scol)
                eng = nc.gpsimd if g < 4 else nc.sync
                eng.dma_start(out_d[128 * g:128 * (g + 1), :], ot[:])

    nc.compile()
    return nc


def _make_consts(s_fc_w, s_fc_b, t_fc_w, t_fc_b, dep_emb, cls_w, cls_b):
    cw = np.asarray(cls_w, np.float32)
    wpack = np.zeros((H, WCOLS), np.float32)
    wpack[:, 0:12] = cw[:, 0:H].T
    wpack[:, 12:24] = cw[:, H:2 * H].T
    wpack[:, 24:36] = cw[:, 2 * H:].T
    wpack[:, 36:84] = np.asarray(dep_emb, np.float32).T
    wpack[:, 84] = np.asarray(s_fc_b, np.float32)
    wpack[:, 85] = np.asarray(t_fc_b, np.float32)
    wpack[0, 86:98] = np.asarray(cls_b, np.float32)
    wpack[0, 98] = 1.0
    # rep128[c, 32s+12i+c] = 1
    for s in range(4):
        for i in range(2):
            for c in range(C):
                wpack[c, 99 + 32 * s + 12 * i + c] = 1.0
    return {"wpk": wpack.astype(np.float16)}


_COLBASE = None


def _marshal_core(n, input_tensor, dg, consts):
    global _COLBASE
    b, half = n // 2, n % 2
    i0 = half * RPC

    xbT = np.roll(input_tensor[b].T, -i0, axis=1)  # [768, 256] rotated j
    sfw = consts["_sfw"]
    tfw = consts["_tfw"]
    big = np.concatenate(
        [sfw[0:128], sfw[128:256], tfw[0:128], tfw[128:256]]
        + [xbT[128 * m:128 * (m + 1)] for m in range(6)],
        axis=1,
    ).astype(np.float16)  # [128, 4608]

    dgc = np.roll(np.asarray(dg[b, i0:i0 + RPC]), -i0, axis=1)  # [128, 256]
    if _COLBASE is None:
        # i_loc r = 32s + 16i + 2g + h -> matmul m = 4g + s,
        # oh col = 512m + 256h + j', oh row block = 48i
        r = np.arange(RPC)
        s_, i_, g_, h_ = r // 32, (r % 32) // 16, (r % 16) // 2, r % 2
        _COLBASE = (
            (512 * (4 * g_ + s_) + 256 * h_)[:, None] + np.arange(L)[None, :],
            (64 * i_)[:, None],
        )
    rows = _COLBASE[1] + dgc
    oh = np.zeros((112, 64 * L), np.uint8)
    oh[rows.ravel(), _COLBASE[0].ravel()] = FP8_ONE

    return {"wpk": consts["wpk"], "big": big, "oh": oh}


def _assemble_core(raw, i0):
    # raw [1024, 512] f16: row = 128g + 32s + 12i + c (24:32 pad),
    # col = 256h + j'; i_loc = 32s + 16i + 2g + h,
    # col j' holds global j = (j'+i0) % L
    arr = raw.reshape(8, 4, 32, 2, L)[:, :, :24]    # (g, s, ic, h, j)
    arr = arr.reshape(8, 4, 2, C, 2, L).transpose(1, 2, 0, 4, 3, 5)
    arr = arr.reshape(RPC, C, L)
    return np.roll(arr, i0, axis=2).astype(np.float32)


def kernel(input_tensor, dependency_graph, s_fc_w, s_fc_b, t_fc_w, t_fc_b,
           dep_emb, cls_w, cls_b):
    global _PROGRAM
    from concourse.bass_utils import run_bass_kernel_spmd

    input_tensor = np.asarray(input_tensor, dtype=np.float32)
    dg = np.asarray(dependency_graph)

    consts = _make_consts(s_fc_w, s_fc_b, t_fc_w, t_fc_b, dep_emb,
                          cls_w, cls_b)
    consts["_sfw"] = np.asarray(s_fc_w, np.float32)
    consts["_tfw"] = np.asarray(t_fc_w, np.float32)

    if _PROGRAM is None:
        _PROGRAM = build_program()
    nc = _PROGRAM

    in_maps = [_marshal_core(n, input_tensor, dg, consts) for n in range(NCORES)]
    trace = bool(int(os.environ.get("KERNEL_PROFILE", "0")))
    res = run_bass_kernel_spmd(
        nc, in_maps, core_ids=list(range(NCORES)), trace=trace
    )
    if trace and res.exec_time_ns is not None:
        print(f"HW exec time: {res.exec_time_ns} ns")

    out = np.empty((B, L, C, L), dtype=np.float32)
    for n in range(NCORES):
        b, half = n // 2, n % 2
        i0 = half * RPC
        out[b, i0:i0 + RPC] = _assemble_core(res.results[n]["out"], i0)
    return out


# revision 25
# speedup vs baseline: 2.1455x; 1.0575x over previous
"""Trainium2 Bass kernel for nn_DependencyLinearLayer.

Math (collapsed-H reformulation of the reference):
  out[b,i,c,j] = DL[dg[b,i,j], c] + s_log[b,i,c] + t_log[b,j,c] + bias[c]
where
  DL        = dep_emb @ w_d.T                     [48, 12]
  s_log     = x @ (w_s @ s_fc_w).T                (combined-weight form)
  t_log     = x @ (w_t @ t_fc_w).T
  bias      = cls_b + w_s@s_fc_b + w_t@t_fc_b     (folded into s_log)
  w_s, w_t, w_d = cls_w[:, :H], cls_w[:, H:2H], cls_w[:, 2H:]

The per-element 48-entry lookup DL[dg] runs on the TENSOR engine as a
one-hot matmul: the host marshals dg into a one-hot fp8 matrix (pure
index->bit-pattern encoding, no float math) and the PE contracts it with
a block-diagonal fp16 [96, 24] stationary diag(DL, DL), gathering two
i-rows per pass at 1 column/cycle.  All floating-point math (projections,
DL, broadcast adds) runs on-device (PE/ACT/DVE); the host only
reshapes/shards inputs.

Sharding: 8 cores; core n handles batch b = n//2 and i-rows
[128*(n%2), 128*(n%2)+128).  The j axis is rotated by i0 per core so the
s-projection reads a fixed column slice of the shared x^T tile.
"""

import os
import sys

import numpy as np

for _p in ("/opt/trn_rl_repo",):
    if _p not in sys.path:
        sys.path.insert(0, _p)

import concourse.bass as bass  # noqa: E402
import concourse.tile as tile  # noqa: E402
from concourse import bacc, mybir  # noqa: E402
from concourse.tile import ScopedClock, add_dep_helper  # noqa: E402

B, L, IN, H, C, NDEP = 4, 256, 768, 256, 12, 48
NCORES = 8
RPC = L // 2      # i-rows per core (128)
NBANK = 8         # PSUM gather banks; each holds 4 slots x 2 col-halves
NM = 32           # gather matmuls per core (one per [24, 512] stripe)
FP8_ONE = 0x38    # fp8 e4m3 bit pattern for 1.0

_MAX_TAIL_WAITS = 1


def _patched_drain_and_barrier(self, tick_clock, wait_clock):
    # The walrus build in this image rejects >1 sync-wait on one CTRL
    # instruction; split the kernel-tail drain waits across nops.
    drain_inst = self.nc.sync.drain()
    wait_clock.add_sem_waits(
        drain_inst.ins, ScopedClock({None: tick_clock.global_clock})
    )
    sync_info = drain_inst.ins.sync_info
    if sync_info is not None and len(sync_info.on_wait) > _MAX_TAIL_WAITS:
        waits = list(sync_info.on_wait)
        sync_info.on_wait = waits[:_MAX_TAIL_WAITS]
        rest = waits[_MAX_TAIL_WAITS:]
        while rest:
            chunk, rest = rest[:_MAX_TAIL_WAITS], rest[_MAX_TAIL_WAITS:]
            nop = self.nc.sync.nop(nofuse=True, hint="tail_drain_split").ins
            nop.sync_info = mybir.SyncInfo(on_wait=chunk, on_update=[])
    self.nc.all_engine_barrier()
    assert self.sems is not None
    popped = self.nc._tile_sem_poison_stack.pop()
    assert popped is self._sem_poison
    self.nc.clear_and_free_semaphores(list(self.sems.allocated().values()))
    self.nc.all_engine_barrier()


tile.TileContext._drain_and_barrier = _patched_drain_and_barrier

_PROGRAM = None

# wpack column layout (per h-half of the H dim):
# 0:12 wsT | 12:24 wtT | 24:36 wdT | 36:84 depT | 84 sfb | 85 tfb |
# 86:98 cls_b row (partition 0 of half 0 only) | 98 one (p0/h0) |
# 99:227 rep128 (partitions 0:12 of half 0 only)
WCOLS = 227


def build_program():
    f16 = mybir.dt.float16
    f32 = mybir.dt.float32
    nc = bacc.Bacc("TRN2", target_bir_lowering=False, debug=False)

    wpk_d = nc.declare_dram_parameter("wpk", [H, WCOLS], f16, isOutput=False)
    # big = [sfw h0|h1 (1536) | tfw h0|h1 (1536) | xbT m0..m5 (1536)]
    big_d = nc.declare_dram_parameter("big", [128, 4608], f16, isOutput=False)
    # one-hot rows: 64*i + d (rows 48:64 and 112:128 are zero padding so
    # the block-diagonal stationary's copies land on 32-aligned partitions)
    oh_d = nc.declare_dram_parameter("oh", [112, 64 * L], mybir.dt.uint8,
                                     isOutput=False)
    # row = 128g + 32s + 12i + c (rows 24:32 of each 32-block are pad),
    # col = 256h + j'
    out_d = nc.declare_dram_parameter("out", [1024, 2 * L], f16, isOutput=True)

    Identity = mybir.ActivationFunctionType.Identity
    Copy = mybir.ActivationFunctionType.Copy

    with tile.TileContext(nc) as tc:
        with (
            tc.tile_pool(name="const", bufs=1) as cp,
            tc.tile_pool(name="opool", bufs=3) as op,
            tc.tile_pool(name="smallp", bufs=1, space="PSUM") as sp,
            tc.tile_pool(name="projp", bufs=1, space="PSUM") as jp,
            tc.tile_pool(name="prept", bufs=1, space="PSUM") as qp,
            tc.tile_pool(name="gp", bufs=5, space="PSUM") as gp,
        ):
            # ---- loads ----
            wpk = cp.tile([128, 2 * WCOLS], f16, tag="wpk")
            with tc.high_priority():
                nc.sync.dma_start(
                    wpk[:].rearrange("p (h q) -> p h q", h=2),
                    wpk_d[:].rearrange("(h p) q -> p h q", h=2),
                )

            def wp(h0, a, b):
                return wpk[:, h0 * WCOLS + a: h0 * WCOLS + b]

            big = cp.tile([128, 4608], f16, tag="big")
            big_dmas = []
            for k in range(3):
                big_dmas.append(nc.sync.dma_start(
                    big[:, 1536 * k:1536 * (k + 1)],
                    big_d[:, 1536 * k:1536 * (k + 1)]))

            def xch(m, w):
                return big[:, 3072 + 256 * m: 3072 + 256 * m + w]

            # the one-hot isn't needed until after the projections; make its
            # transfers yield HBM bandwidth to the critical big/weights load
            oh_t = cp.tile([112, 64 * L], mybir.dt.uint8, tag="oh_t")
            for k in range(4):
                ohd = nc.gpsimd.dma_start(oh_t[:, 4096 * k:4096 * (k + 1)],
                                          oh_d[:, 4096 * k:4096 * (k + 1)])
                add_dep_helper(ohd.ins, big_dmas[2].ins,
                               reason="oh transfers after big load")
            oh8 = oh_t[:].bitcast(mybir.dt.float8e4)

            w_dl = cp.tile([128, 32], f16, tag="w_dl")
            s_all = cp.tile([128, 16], f32, tag="s_all")
            nc.gpsimd.memset(w_dl[:], 0)
            nc.gpsimd.memset(s_all[:], 0)

            # ---- bias row: cls_b + ws@sfb + wt@tfb  [12, 1] ----
            with tc.high_priority():
                pbias = sp.tile([C, 1], f32, tag="small")
                nc.tensor.matmul(pbias[:], wp(0, 0, 12), wp(0, 84, 85),
                                 start=True, stop=False)
                nc.tensor.matmul(pbias[:], wp(1, 0, 12), wp(1, 84, 85),
                                 start=False, stop=False)
                nc.tensor.matmul(pbias[:], wp(0, 12, 24), wp(0, 85, 86),
                                 start=False, stop=False)
                nc.tensor.matmul(pbias[:], wp(1, 12, 24), wp(1, 85, 86),
                                 start=False, stop=False)
                nc.tensor.matmul(pbias[:], wp(0, 86, 98)[0:1, :],
                                 wp(0, 98, 99)[0:1, :],
                                 start=False, stop=True)
                bias_sb = cp.tile([C, 1], f32, tag="bias_sb")
                nc.scalar.copy(bias_sb[:], pbias[:])

                # ---- DL [48, 12] then block-diagonal [96, 24] fp16 ----
                pd2 = sp.tile([NDEP, C], f32, tag="small")
                nc.tensor.matmul(pd2[:], wp(0, 36, 84), wp(0, 24, 36),
                                 start=True, stop=False)
                pd2_mm = nc.tensor.matmul(pd2[:], wp(1, 36, 84), wp(1, 24, 36),
                                          start=False, stop=True)
                nc.vector.tensor_copy(w_dl[0:48, 0:12], pd2[:])
                nc.vector.tensor_copy(w_dl[64:112, 12:24], pd2[:])

            g_tiles = [None] * NBANK

            def gather_bank(g):
                # odd banks: PSUM pre-seeded with the t_log broadcast (their
                # eviction runs on ACT, which can only add a scalar bias)
                gt = gp.tile([128, 512], f32, tag="gt")
                g_tiles[g] = gt
                seed = g % 2 == 1
                if seed:
                    nc.tensor.matmul(
                        gt[:], wp(0, 99, 227)[0:12, :], midbufT[:],
                        start=True, stop=False, skip_group_check=True,
                    )
                for s in range(4):
                    m = 4 * g + s
                    nc.tensor.matmul(
                        gt[32 * s:32 * s + 32, :], w_dl[0:112, :],
                        oh8[:, 512 * m:512 * (m + 1)],
                        start=not seed, stop=(not seed) or (s == 3),
                        skip_group_check=seed,
                        tile_position=(0, 32 * s),
                    )

            # ---- combined weights w2 = [w2sT | w2tT] per IN-chunk ----
            # all 6 chunks accumulate into one PSUM tile; single cast out
            w2_t = cp.tile([128, 6 * 2 * C], f16, tag="w2_t")
            pw = sp.tile([128, 6 * 2 * C], f32, tag="small")
            for m in range(6):
                for h0 in range(2):
                    nc.tensor.matmul(
                        pw[:, 24 * m:24 * m + C],
                        big[:, 768 * h0 + 128 * m: 768 * h0 + 128 * (m + 1)],
                        wp(h0, 0, 12),
                        start=(h0 == 0), stop=(h0 == 1),
                    )
                for h0 in range(2):
                    nc.tensor.matmul(
                        pw[:, 24 * m + C:24 * m + 2 * C],
                        big[:, 1536 + 768 * h0 + 128 * m:
                             1536 + 768 * h0 + 128 * (m + 1)],
                        wp(h0, 12, 24),
                        start=(h0 == 0), stop=(h0 == 1),
                    )
            nc.vector.tensor_copy(w2_t[:], pw[:])

            # ---- projections: s_log [12, 128] first (s_all gates the
            # epilogue), then t_log [12, 256] ----
            ps = jp.tile([C, RPC], f32, tag="proj")
            for m in range(6):
                nc.tensor.matmul(
                    ps[:], w2_t[:, 24 * m:24 * m + 12], xch(m, 128),
                    start=(m == 0), stop=(m == 5),
                )
            slog_t = cp.tile([C, RPC], f32, tag="slog_t")
            nc.scalar.activation(slog_t[:], ps[:], Identity, bias=bias_sb[:])

            pt = jp.tile([C, L], f32, tag="proj")
            for m in range(6):
                nc.tensor.matmul(
                    pt[:], w2_t[:, 24 * m + 12:24 * m + 24], xch(m, 256),
                    start=(m == 0), stop=(m == 5),
                )
            midbufT = cp.tile([C, 2 * L], f16, tag="midbufT")
            nc.scalar.copy(midbufT[:, 0:L], pt[:])
            nc.scalar.copy(midbufT[:, L:2 * L], pt[:])

            # ---- t120 [128, 256]: t_log replicated to (s, i, c) rows ----
            prepT = qp.tile([128, L], f32, tag="prepT")
            nc.tensor.matmul(prepT[:], wp(0, 99, 227)[0:12, :],
                             midbufT[:, 0:L], start=True, stop=True)
            t120 = cp.tile([128, L], f32, tag="t120")
            nc.vector.tensor_copy(t120[:], prepT[:])

            # ---- s_all [128, 16]: s_all[32s+12i+c, 2g+h] =
            #      slog_t[c, 16g+4s+2h+i], one strided DMA ----
            # i_loc(g,s,h,i) = 32s + 16i + 2g + h, so per-(s,i) the 16
            # (g,h) scalars are a contiguous slog_t slice.
            # s_all[32s+12i+c, 2g+h] = slog_t[c, 32s+16i+2g+h]
            for s in range(4):
                for i in range(2):
                    eng = nc.sync if s < 2 else nc.scalar
                    eng.dma_start(
                        s_all[32 * s + 12 * i:32 * s + 12 * i + 12, :],
                        slog_t[0:12, 32 * s + 16 * i:32 * s + 16 * i + 16],
                    )

            # ---- gather banks + epilogue (out = gt + t120 + s) ----
            Add = mybir.AluOpType.add
            for g in range(NBANK):
                gather_bank(g)
            for g in range(NBANK):
                gt = g_tiles[g]
                ot = op.tile([128, 512], f16, tag="ot")
                for h in range(2):
                    dst = ot[:, 256 * h:256 * (h + 1)]
                    srcp = gt[:, 256 * h:256 * (h + 1)]
                    scol = s_all[:, 2 * g + h:2 * g + h + 1]
                    if g % 2 == 0:
                        nc.vector.affine_then_add(dst, srcp, t120[:], 1.0,
                                                  scol)
                    else:
                        nc.scalar.activation(dst, srcp, Identity, bias=scol)
                eng = nc.gpsimd if g < 4 else nc.sync
                eng.dma_start(out_d[128 * g:128 * (g + 1), :], ot[:])

    nc.compile()
    return nc


def _make_consts(s_fc_w, s_fc_b, t_fc_w, t_fc_b, dep_emb, cls_w, cls_b):
    cw = np.asarray(cls_w, np.float32)
    wpack = np.zeros((H, WCOLS), np.float32)
    wpack[:, 0:12] = cw[:, 0:H].T
    wpack[:, 12:24] = cw[:, H:2 * H].T
    wpack[:, 24:36] = cw[:, 2 * H:].T
    wpack[:, 36:84] = np.asarray(dep_emb, np.float32).T
    wpack[:, 84] = np.asarray(s_fc_b, np.float32)
    wpack[:, 85] = np.asarray(t_fc_b, np.float32)
    wpack[0, 86:98] = np.asarray(cls_b, np.float32)
    wpack[0, 98] = 1.0
    # rep128[c, 32s+12i+c] = 1
    for s in range(4):
        for i in range(2):
            for c in range(C):
                wpack[c, 99 + 32 * s + 12 * i + c] = 1.0
    return {"wpk": wpack.astype(np.float16)}


_COLBASE = None


def _marshal_core(n, input_tensor, dg, consts):
    global _COLBASE
    b, half = n // 2, n % 2
    i0 = half * RPC

    xbT = np.roll(input_tensor[b].T, -i0, axis=1)  # [768, 256] rotated j
    sfw = consts["_sfw"]
    tfw = consts["_tfw"]
    big = np.concatenate(
        [sfw[0:128], sfw[128:256], tfw[0:128], tfw[128:256]]
        + [xbT[128 * m:128 * (m + 1)] for m in range(6)],
        axis=1,
    ).astype(np.float16)  # [128, 4608]

    dgc = np.roll(np.asarray(dg[b, i0:i0 + RPC]), -i0, axis=1)  # [128, 256]
    if _COLBASE is None:
        # i_loc r = 32s + 16i + 2g + h -> matmul m = 4g + s,
        # oh col = 512m + 256h + j', oh row block = 48i
        r = np.arange(RPC)
        s_, i_, g_, h_ = r // 32, (r % 32) // 16, (r % 16) // 2, r % 2
        _COLBASE = (
            (512 * (4 * g_ + s_) + 256 * h_)[:, None] + np.arange(L)[None, :],
            (64 * i_)[:, None],
        )
    rows = _COLBASE[1] + dgc
    oh = np.zeros((112, 64 * L), np.uint8)
    oh[rows.ravel(), _COLBASE[0].ravel()] = FP8_ONE

    return {"wpk": consts["wpk"], "big": big, "oh": oh}


def _assemble_core(raw, i0):
    # raw [1024, 512] f16: row = 128g + 32s + 12i + c (24:32 pad),
    # col = 256h + j'; i_loc = 32s + 16i + 2g + h,
    # col j' holds global j = (j'+i0) % L
    arr = raw.reshape(8, 4, 32, 2, L)[:, :, :24]    # (g, s, ic, h, j)
    arr = arr.reshape(8, 4, 2, C, 2, L).transpose(1, 2, 0, 4, 3, 5)
    arr = arr.reshape(RPC, C, L)
    return np.roll(arr, i0, axis=2).astype(np.float32)


def kernel(input_tensor, dependency_graph, s_fc_w, s_fc_b, t_fc_w, t_fc_b,
           dep_emb, cls_w, cls_b):
    global _PROGRAM
    from concourse.bass_utils import run_bass_kernel_spmd

    input_tensor = np.asarray(input_tensor, dtype=np.float32)
    dg = np.asarray(dependency_graph)

    consts = _make_consts(s_fc_w, s_fc_b, t_fc_w, t_fc_b, dep_emb,
                          cls_w, cls_b)
    consts["_sfw"] = np.asarray(s_fc_w, np.float32)
    consts["_tfw"] = np.asarray(t_fc_w, np.float32)

    if _PROGRAM is None:
        _PROGRAM = build_program()
    nc = _PROGRAM

    in_maps = [_marshal_core(n, input_tensor, dg, consts) for n in range(NCORES)]
    trace = bool(int(os.environ.get("KERNEL_PROFILE", "0")))
    res = run_bass_kernel_spmd(
        nc, in_maps, core_ids=list(range(NCORES)), trace=trace
    )
    if trace and res.exec_time_ns is not None:
        print(f"HW exec time: {res.exec_time_ns} ns")

    out = np.empty((B, L, C, L), dtype=np.float32)
    for n in range(NCORES):
        b, half = n // 2, n % 2
        i0 = half * RPC
        out[b, i0:i0 + RPC] = _assemble_core(res.results[n]["out"], i0)
    return out


# revision 27
# speedup vs baseline: 2.1456x; 1.0001x over previous
"""Trainium2 Bass kernel for nn_DependencyLinearLayer.

Math (collapsed-H reformulation of the reference):
  out[b,i,c,j] = DL[dg[b,i,j], c] + s_log[b,i,c] + t_log[b,j,c] + bias[c]
where
  DL        = dep_emb @ w_d.T                     [48, 12]
  s_log     = x @ (w_s @ s_fc_w).T                (combined-weight form)
  t_log     = x @ (w_t @ t_fc_w).T
  bias      = cls_b + w_s@s_fc_b + w_t@t_fc_b     (folded into s_log)
  w_s, w_t, w_d = cls_w[:, :H], cls_w[:, H:2H], cls_w[:, 2H:]

The per-element 48-entry lookup DL[dg] runs on the TENSOR engine as a
one-hot matmul: the host marshals dg into a one-hot fp8 matrix (pure
index->bit-pattern encoding, no float math) and the PE contracts it with
a block-diagonal fp16 [96, 24] stationary diag(DL, DL), gathering two
i-rows per pass at 1 column/cycle.  All floating-point math (projections,
DL, broadcast adds) runs on-device (PE/ACT/DVE); the host only
reshapes/shards inputs.

Sharding: 8 cores; core n handles batch b = n//2 and i-rows
[128*(n%2), 128*(n%2)+128).  The j axis is rotated by i0 per core so the
s-projection reads a fixed column slice of the shared x^T tile.
"""

import os
import sys

import numpy as np

for _p in ("/opt/trn_rl_repo",):
    if _p not in sys.path:
        sys.path.insert(0, _p)

import concourse.bass as bass  # noqa: E402
import concourse.tile as tile  # noqa: E402
from concourse import bacc, mybir  # noqa: E402
from concourse.tile import ScopedClock, add_dep_helper  # noqa: E402

B, L, IN, H, C, NDEP = 4, 256, 768, 256, 12, 48
NCORES = 8
RPC = L // 2      # i-rows per core (128)
NBANK = 8         # PSUM gather banks; each holds 4 slots x 2 col-halves
NM = 32           # gather matmuls per core (one per [24, 512] stripe)
FP8_ONE = 0x38    # fp8 e4m3 bit pattern for 1.0

_MAX_TAIL_WAITS = 1


def _patched_drain_and_barrier(self, tick_clock, wait_clock):
    # The walrus build in this image rejects >1 sync-wait on one CTRL
    # instruction; split the kernel-tail drain waits across nops.
    drain_inst = self.nc.sync.drain()
    wait_clock.add_sem_waits(
        drain_inst.ins, ScopedClock({None: tick_clock.global_clock})
    )
    sync_info = drain_inst.ins.sync_info
    if sync_info is not None and len(sync_info.on_wait) > _MAX_TAIL_WAITS:
        waits = list(sync_info.on_wait)
        sync_info.on_wait = waits[:_MAX_TAIL_WAITS]
        rest = waits[_MAX_TAIL_WAITS:]
        while rest:
            chunk, rest = rest[:_MAX_TAIL_WAITS], rest[_MAX_TAIL_WAITS:]
            nop = self.nc.sync.nop(nofuse=True, hint="tail_drain_split").ins
            nop.sync_info = mybir.SyncInfo(on_wait=chunk, on_update=[])
    self.nc.all_engine_barrier()
    assert self.sems is not None
    popped = self.nc._tile_sem_poison_stack.pop()
    assert popped is self._sem_poison
    self.nc.clear_and_free_semaphores(list(self.sems.allocated().values()))
    self.nc.all_engine_barrier()


tile.TileContext._drain_and_barrier = _patched_drain_and_barrier

_PROGRAM = None

# wpack column layout (per h-half of the H dim):
# 0:12 wsT | 12:24 wtT | 24:36 wdT | 36:84 depT | 84 sfb | 85 tfb |
# 86:98 cls_b row (partition 0 of half 0 only) | 98 one (p0/h0) |
# 99:227 rep128 (partitions 0:12 of half 0 only)
WCOLS = 227


def build_program():
    f16 = mybir.dt.float16
    f32 = mybir.dt.float32
    nc = bacc.Bacc("TRN2", target_bir_lowering=False, debug=False)

    wpk_d = nc.declare_dram_parameter("wpk", [H, WCOLS], f16, isOutput=False)
    # big = [sfw h0|h1 (1536) | tfw h0|h1 (1536) | xbT m0..m5 (1536)]
    big_d = nc.declare_dram_parameter("big", [128, 4608], f16, isOutput=False)
    # one-hot rows: 64*i + d (rows 48:64 and 112:128 are zero padding so
    # the block-diagonal stationary's copies land on 32-aligned partitions)
    oh_d = nc.declare_dram_parameter("oh", [112, 64 * L], mybir.dt.uint8,
                                     isOutput=False)
    # row = 128g + 32s + 12i + c (rows 24:32 of each 32-block are pad),
    # col = 256h + j'
    out_d = nc.declare_dram_parameter("out", [1024, 2 * L], f16, isOutput=True)

    Identity = mybir.ActivationFunctionType.Identity
    Copy = mybir.ActivationFunctionType.Copy

    with tile.TileContext(nc) as tc:
        with (
            tc.tile_pool(name="const", bufs=1) as cp,
            tc.tile_pool(name="opool", bufs=3) as op,
            tc.tile_pool(name="smallp", bufs=1, space="PSUM") as sp,
            tc.tile_pool(name="projp", bufs=1, space="PSUM") as jp,
            tc.tile_pool(name="prept", bufs=1, space="PSUM") as qp,
            tc.tile_pool(name="gp", bufs=5, space="PSUM") as gp,
        ):
            # ---- loads ----
            wpk = cp.tile([128, 2 * WCOLS], f16, tag="wpk")
            with tc.high_priority():
                nc.sync.dma_start(
                    wpk[:].rearrange("p (h q) -> p h q", h=2),
                    wpk_d[:].rearrange("(h p) q -> p h q", h=2),
                )

            def wp(h0, a, b):
                return wpk[:, h0 * WCOLS + a: h0 * WCOLS + b]

            big = cp.tile([128, 4608], f16, tag="big")
            big_dmas = []
            for k in range(3):
                big_dmas.append(nc.sync.dma_start(
                    big[:, 1536 * k:1536 * (k + 1)],
                    big_d[:, 1536 * k:1536 * (k + 1)]))

            def xch(m, w):
                return big[:, 3072 + 256 * m: 3072 + 256 * m + w]

            # the one-hot isn't needed until after the projections; make its
            # transfers yield HBM bandwidth to the critical big/weights load
            oh_t = cp.tile([112, 64 * L], mybir.dt.uint8, tag="oh_t")
            for k in range(4):
                ohd = nc.gpsimd.dma_start(oh_t[:, 4096 * k:4096 * (k + 1)],
                                          oh_d[:, 4096 * k:4096 * (k + 1)])
                add_dep_helper(ohd.ins, big_dmas[2].ins,
                               reason="oh transfers after big load")
            oh8 = oh_t[:].bitcast(mybir.dt.float8e4)

            w_dl = cp.tile([128, 32], f16, tag="w_dl")
            s_all = cp.tile([128, 16], f32, tag="s_all")
            nc.gpsimd.memset(w_dl[:], 0)
            nc.gpsimd.memset(s_all[:], 0)

            # ---- bias row: cls_b + ws@sfb + wt@tfb  [12, 1] ----
            with tc.high_priority():
                pbias = sp.tile([C, 1], f32, tag="small")
                nc.tensor.matmul(pbias[:], wp(0, 0, 12), wp(0, 84, 85),
                                 start=True, stop=False)
                nc.tensor.matmul(pbias[:], wp(1, 0, 12), wp(1, 84, 85),
                                 start=False, stop=False)
                nc.tensor.matmul(pbias[:], wp(0, 12, 24), wp(0, 85, 86),
                                 start=False, stop=False)
                nc.tensor.matmul(pbias[:], wp(1, 12, 24), wp(1, 85, 86),
                                 start=False, stop=False)
                nc.tensor.matmul(pbias[:], wp(0, 86, 98)[0:1, :],
                                 wp(0, 98, 99)[0:1, :],
                                 start=False, stop=True)
                bias_sb = cp.tile([C, 1], f32, tag="bias_sb")
                nc.scalar.copy(bias_sb[:], pbias[:])

                # ---- DL [48, 12] then block-diagonal [96, 24] fp16 ----
                pd2 = sp.tile([NDEP, C], f32, tag="small")
                nc.tensor.matmul(pd2[:], wp(0, 36, 84), wp(0, 24, 36),
                                 start=True, stop=False)
                pd2_mm = nc.tensor.matmul(pd2[:], wp(1, 36, 84), wp(1, 24, 36),
                                          start=False, stop=True)
                nc.vector.tensor_copy(w_dl[0:48, 0:12], pd2[:])
                nc.vector.tensor_copy(w_dl[64:112, 12:24], pd2[:])

            g_tiles = [None] * NBANK

            ACT_BANKS = (2, 3, 4, 5)  # evicted on ACT; PSUM seeded with t

            def gather_bank(g):
                gt = gp.tile([128, 512], f32, tag="gt")
                g_tiles[g] = gt
                seed = g in ACT_BANKS
                if seed:
                    nc.tensor.matmul(
                        gt[:], wp(0, 99, 227)[0:12, :], midbufT[:],
                        start=True, stop=False, skip_group_check=True,
                    )
                for s in range(4):
                    m = 4 * g + s
                    nc.tensor.matmul(
                        gt[32 * s:32 * s + 32, :], w_dl[0:112, :],
                        oh8[:, 512 * m:512 * (m + 1)],
                        start=not seed, stop=(not seed) or (s == 3),
                        skip_group_check=seed,
                        tile_position=(0, 32 * s),
                    )

            # ---- combined weights w2 = [w2sT | w2tT] per IN-chunk ----
            # all 6 chunks accumulate into one PSUM tile; single cast out
            w2_t = cp.tile([128, 6 * 2 * C], f16, tag="w2_t")
            pw = sp.tile([128, 6 * 2 * C], f32, tag="small")
            for m in range(6):
                for h0 in range(2):
                    nc.tensor.matmul(
                        pw[:, 24 * m:24 * m + C],
                        big[:, 768 * h0 + 128 * m: 768 * h0 + 128 * (m + 1)],
                        wp(h0, 0, 12),
                        start=(h0 == 0), stop=(h0 == 1),
                    )
                for h0 in range(2):
                    nc.tensor.matmul(
                        pw[:, 24 * m + C:24 * m + 2 * C],
                        big[:, 1536 + 768 * h0 + 128 * m:
                             1536 + 768 * h0 + 128 * (m + 1)],
                        wp(h0, 12, 24),
                        start=(h0 == 0), stop=(h0 == 1),
                    )
            nc.vector.tensor_copy(w2_t[:], pw[:])

            # ---- projections: s_log [12, 128] first (s_all gates the
            # epilogue), then t_log [12, 256] ----
            ps = jp.tile([C, RPC], f32, tag="proj")
            for m in range(6):
                nc.tensor.matmul(
                    ps[:], w2_t[:, 24 * m:24 * m + 12], xch(m, 128),
                    start=(m == 0), stop=(m == 5),
                )
            slog_t = cp.tile([C, RPC], f32, tag="slog_t")
            nc.scalar.activation(slog_t[:], ps[:], Identity, bias=bias_sb[:])

            # i_loc(g,s,h,i) = 32s + 16i + 2g + h, so per-(s,i) the 16
            # (g,h) scalars are a contiguous slog_t slice.
            # s_all[32s+12i+c, 2g+h] = slog_t[c, 32s+16i+2g+h]
            sall_engs = [nc.sync, nc.gpsimd, nc.sync, nc.gpsimd,
                         nc.sync, nc.gpsimd, nc.scalar, nc.scalar]
            for s in range(4):
                for i in range(2):
                    sall_engs[2 * s + i].dma_start(
                        s_all[32 * s + 12 * i:32 * s + 12 * i + 12, :],
                        slog_t[0:12, 32 * s + 16 * i:32 * s + 16 * i + 16],
                    )

            pt = jp.tile([C, L], f32, tag="proj")
            for m in range(6):
                nc.tensor.matmul(
                    pt[:], w2_t[:, 24 * m + 12:24 * m + 24], xch(m, 256),
                    start=(m == 0), stop=(m == 5),
                )
            midbufT = cp.tile([C, 2 * L], f16, tag="midbufT")
            nc.scalar.copy(midbufT[:, 0:L], pt[:])
            nc.scalar.copy(midbufT[:, L:2 * L], pt[:])

            # unseeded banks can run while midbufT/prepT are still in flight
            gather_bank(0)
            gather_bank(1)

            # ---- t120 [128, 256]: t_log replicated to (s, i, c) rows ----
            prepT = qp.tile([128, L], f32, tag="prepT")
            nc.tensor.matmul(prepT[:], wp(0, 99, 227)[0:12, :],
                             midbufT[:, 0:L], start=True, stop=True)
            t120 = cp.tile([128, L], f32, tag="t120")
            nc.vector.tensor_copy(t120[:], prepT[:])

            # ---- s_all [128, 16]: s_all[32s+12i+c, 2g+h] =
            #      slog_t[c, 16g+4s+2h+i], one strided DMA ----
            # ---- gather banks + epilogue (out = gt + t120 + s) ----
            for g in range(2, NBANK):
                gather_bank(g)
            for g in range(NBANK):
                gt = g_tiles[g]
                ot = op.tile([128, 512], f16, tag="ot")
                for h in range(2):
                    dst = ot[:, 256 * h:256 * (h + 1)]
                    srcp = gt[:, 256 * h:256 * (h + 1)]
                    scol = s_all[:, 2 * g + h:2 * g + h + 1]
                    if g in ACT_BANKS:
                        nc.scalar.activation(dst, srcp, Identity, bias=scol)
                    else:
                        nc.vector.affine_then_add(dst, srcp, t120[:], 1.0,
                                                  scol)
                eng = nc.gpsimd if g < 4 else nc.sync
                eng.dma_start(out_d[128 * g:128 * (g + 1), :], ot[:])

    nc.compile()
    return nc


def _make_consts(s_fc_w, s_fc_b, t_fc_w, t_fc_b, dep_emb, cls_w, cls_b):
    cw = np.asarray(cls_w, np.float32)
    wpack = np.zeros((H, WCOLS), np.float32)
    wpack[:, 0:12] = cw[:, 0:H].T
    wpack[:, 12:24] = cw[:, H:2 * H].T
    wpack[:, 24:36] = cw[:, 2 * H:].T
    wpack[:, 36:84] = np.asarray(dep_emb, np.float32).T
    wpack[:, 84] = np.asarray(s_fc_b, np.float32)
    wpack[:, 85] = np.asarray(t_fc_b, np.float32)
    wpack[0, 86:98] = np.asarray(cls_b, np.float32)
    wpack[0, 98] = 1.0
    # rep128[c, 32s+12i+c] = 1
    for s in range(4):
        for i in range(2):
            for c in range(C):
                wpack[c, 99 + 32 * s + 12 * i + c] = 1.0
    return {"wpk": wpack.astype(np.float16)}


_COLBASE = None


def _marshal_core(n, input_tensor, dg, consts):
    global _COLBASE
    b, half = n // 2, n % 2
    i0 = half * RPC

    xbT = np.roll(input_tensor[b].T, -i0, axis=1)  # [768, 256] rotated j
    sfw = consts["_sfw"]
    tfw = consts["_tfw"]
    big = np.concatenate(
        [sfw[0:128], sfw[128:256], tfw[0:128], tfw[128:256]]
        + [xbT[128 * m:128 * (m + 1)] for m in range(6)],
        axis=1,
    ).astype(np.float16)  # [128, 4608]

    dgc = np.roll(np.asarray(dg[b, i0:i0 + RPC]), -i0, axis=1)  # [128, 256]
    if _COLBASE is None:
        # i_loc r = 32s + 16i + 2g + h -> matmul m = 4g + s,
        # oh col = 512m + 256h + j', oh row block = 48i
        r = np.arange(RPC)
        s_, i_, g_, h_ = r // 32, (r % 32) // 16, (r % 16) // 2, r % 2
        _COLBASE = (
            (512 * (4 * g_ + s_) + 256 * h_)[:, None] + np.arange(L)[None, :],
            (64 * i_)[:, None],
        )
    rows = _COLBASE[1] + dgc
    oh = np.zeros((112, 64 * L), np.uint8)
    oh[rows.ravel(), _COLBASE[0].ravel()] = FP8_ONE

    return {"wpk": consts["wpk"], "big": big, "oh": oh}


def _assemble_core(raw, i0):
    # raw [1024, 512] f16: row = 128g + 32s + 12i + c (24:32 pad),
    # col = 256h + j'; i_loc = 32s + 16i + 2g + h,
    # col j' holds global j = (j'+i0) % L
    arr = raw.reshape(8, 4, 32, 2, L)[:, :, :24]    # (g, s, ic, h, j)
    arr = arr.reshape(8, 4, 2, C, 2, L).transpose(1, 2, 0, 4, 3, 5)
    arr = arr.reshape(RPC, C, L)
    return np.roll(arr, i0, axis=2).astype(np.float32)


def kernel(input_tensor, dependency_graph, s_fc_w, s_fc_b, t_fc_w, t_fc_b,
           dep_emb, cls_w, cls_b):
    global _PROGRAM
    from concourse.bass_utils import run_bass_kernel_spmd

    input_tensor = np.asarray(input_tensor, dtype=np.float32)
    dg = np.asarray(dependency_graph)

    consts = _make_consts(s_fc_w, s_fc_b, t_fc_w, t_fc_b, dep_emb,
                          cls_w, cls_b)
    consts["_sfw"] = np.asarray(s_fc_w, np.float32)
    consts["_tfw"] = np.asarray(t_fc_w, np.float32)

    if _PROGRAM is None:
        _PROGRAM = build_program()
    nc = _PROGRAM

    in_maps = [_marshal_core(n, input_tensor, dg, consts) for n in range(NCORES)]
    trace = bool(int(os.environ.get("KERNEL_PROFILE", "0")))
    res = run_bass_kernel_spmd(
        nc, in_maps, core_ids=list(range(NCORES)), trace=trace
    )
    if trace and res.exec_time_ns is not None:
        print(f"HW exec time: {res.exec_time_ns} ns")

    out = np.empty((B, L, C, L), dtype=np.float32)
    for n in range(NCORES):
        b, half = n // 2, n % 2
        i0 = half * RPC
        out[b, i0:i0 + RPC] = _assemble_core(res.results[n]["out"], i0)
    return out


# revision 30
# speedup vs baseline: 2.1627x; 1.0080x over previous
"""Trainium2 Bass kernel for nn_DependencyLinearLayer.

Math (collapsed-H reformulation of the reference):
  out[b,i,c,j] = DL[dg[b,i,j], c] + s_log[b,i,c] + t_log[b,j,c] + bias[c]
where
  DL        = dep_emb @ w_d.T                     [48, 12]
  s_log     = x @ (w_s @ s_fc_w).T                (combined-weight form)
  t_log     = x @ (w_t @ t_fc_w).T
  bias      = cls_b + w_s@s_fc_b + w_t@t_fc_b     (folded into s_log)
  w_s, w_t, w_d = cls_w[:, :H], cls_w[:, H:2H], cls_w[:, 2H:]

The per-element 48-entry lookup DL[dg] runs on the TENSOR engine as a
one-hot matmul: the host marshals dg into a one-hot fp8 matrix (pure
index->bit-pattern encoding, no float math) and the PE contracts it with
a block-diagonal fp16 [96, 24] stationary diag(DL, DL), gathering two
i-rows per pass at 1 column/cycle.  All floating-point math (projections,
DL, broadcast adds) runs on-device (PE/ACT/DVE); the host only
reshapes/shards inputs.

Sharding: 8 cores; core n handles batch b = n//2 and i-rows
[128*(n%2), 128*(n%2)+128).  The j axis is rotated by i0 per core so the
s-projection reads a fixed column slice of the shared x^T tile.
"""

import os
import sys

import numpy as np

for _p in ("/opt/trn_rl_repo",):
    if _p not in sys.path:
        sys.path.insert(0, _p)

import concourse.bass as bass  # noqa: E402
import concourse.tile as tile  # noqa: E402
from concourse import bacc, mybir  # noqa: E402
from concourse.tile import ScopedClock, add_dep_helper  # noqa: E402

B, L, IN, H, C, NDEP = 4, 256, 768, 256, 12, 48
NCORES = 8
RPC = L // 2      # i-rows per core (128)
NBANK = 8         # PSUM gather banks; each holds 4 slots x 2 col-halves
NM = 32           # gather matmuls per core (one per [24, 512] stripe)
FP8_ONE = 0x38    # fp8 e4m3 bit pattern for 1.0

_MAX_TAIL_WAITS = 1


def _patched_drain_and_barrier(self, tick_clock, wait_clock):
    # The walrus build in this image rejects >1 sync-wait on one CTRL
    # instruction; split the kernel-tail drain waits across nops.
    drain_inst = self.nc.sync.drain()
    wait_clock.add_sem_waits(
        drain_inst.ins, ScopedClock({None: tick_clock.global_clock})
    )
    sync_info = drain_inst.ins.sync_info
    if sync_info is not None and len(sync_info.on_wait) > _MAX_TAIL_WAITS:
        waits = list(sync_info.on_wait)
        sync_info.on_wait = waits[:_MAX_TAIL_WAITS]
        rest = waits[_MAX_TAIL_WAITS:]
        while rest:
            chunk, rest = rest[:_MAX_TAIL_WAITS], rest[_MAX_TAIL_WAITS:]
            nop = self.nc.sync.nop(nofuse=True, hint="tail_drain_split").ins
            nop.sync_info = mybir.SyncInfo(on_wait=chunk, on_update=[])
    self.nc.all_engine_barrier()
    assert self.sems is not None
    popped = self.nc._tile_sem_poison_stack.pop()
    assert popped is self._sem_poison
    self.nc.clear_and_free_semaphores(list(self.sems.allocated().values()))
    self.nc.all_engine_barrier()


tile.TileContext._drain_and_barrier = _patched_drain_and_barrier

_PROGRAM = None

# wpack column layout (per h-half of the H dim):
# 0:12 wsT | 12:24 wtT | 24:36 wdT | 36:84 depT | 84 sfb | 85 tfb |
# 86:98 cls_b row (partition 0 of half 0 only) | 98 one (p0/h0) |
# 99:227 rep128 (partitions 0:12 of half 0 only)
WCOLS = 227


def build_program():
    f16 = mybir.dt.float16
    f32 = mybir.dt.float32
    nc = bacc.Bacc("TRN2", target_bir_lowering=False, debug=False)

    wpk_d = nc.declare_dram_parameter("wpk", [H, WCOLS], f16, isOutput=False)
    # big = [sfw h0|h1 (1536) | xbT m0..m5 (1536) | tfw h0|h1 (1536)]
    big_d = nc.declare_dram_parameter("big", [128, 4608], f16, isOutput=False)
    # one-hot rows: 64*i + d (rows 48:64 and 112:128 are zero padding so
    # the block-diagonal stationary's copies land on 32-aligned partitions)
    oh_d = nc.declare_dram_parameter("oh", [112, 64 * L], mybir.dt.uint8,
                                     isOutput=False)
    # row = 128g + 32s + 12i + c (rows 24:32 of each 32-block are pad),
    # col = 256h + j'
    out_d = nc.declare_dram_parameter("out", [1024, 2 * L], f16, isOutput=True)

    Identity = mybir.ActivationFunctionType.Identity
    Copy = mybir.ActivationFunctionType.Copy

    with tile.TileContext(nc) as tc:
        with (
            tc.tile_pool(name="const", bufs=1) as cp,
            tc.tile_pool(name="opool", bufs=3) as op,
            tc.tile_pool(name="smallp", bufs=1, space="PSUM") as sp,
            tc.tile_pool(name="projp", bufs=1, space="PSUM") as jp,
            tc.tile_pool(name="prept", bufs=1, space="PSUM") as qp,
            tc.tile_pool(name="gp", bufs=5, space="PSUM") as gp,
        ):
            # ---- loads ----
            wpk = cp.tile([128, 2 * WCOLS], f16, tag="wpk")
            with tc.high_priority():
                nc.sync.dma_start(
                    wpk[:].rearrange("p (h q) -> p h q", h=2),
                    wpk_d[:].rearrange("(h p) q -> p h q", h=2),
                )

            def wp(h0, a, b):
                return wpk[:, h0 * WCOLS + a: h0 * WCOLS + b]

            big = cp.tile([128, 4608], f16, tag="big")
            big_dmas = []
            for k in range(3):
                big_dmas.append(nc.sync.dma_start(
                    big[:, 1536 * k:1536 * (k + 1)],
                    big_d[:, 1536 * k:1536 * (k + 1)]))

            def xch(m, w):
                return big[:, 1536 + 256 * m: 1536 + 256 * m + w]

            def sfch(m, h0):
                return big[:, 768 * h0 + 128 * m: 768 * h0 + 128 * (m + 1)]

            def tfch(m, h0):
                return big[:, 3072 + 768 * h0 + 128 * m:
                           3072 + 768 * h0 + 128 * (m + 1)]

            # the one-hot isn't needed until after the projections; make its
            # transfers yield HBM bandwidth to the critical big/weights load
            oh_t = cp.tile([112, 64 * L], mybir.dt.uint8, tag="oh_t")
            for k in range(4):
                ohd = nc.gpsimd.dma_start(oh_t[:, 4096 * k:4096 * (k + 1)],
                                          oh_d[:, 4096 * k:4096 * (k + 1)])
                add_dep_helper(ohd.ins, big_dmas[2].ins,
                               reason="oh transfers after big load")
            oh8 = oh_t[:].bitcast(mybir.dt.float8e4)

            w_dl = cp.tile([128, 32], f16, tag="w_dl")
            s_all = cp.tile([128, 16], f32, tag="s_all")
            nc.gpsimd.memset(w_dl[:], 0)
            nc.gpsimd.memset(s_all[:], 0)

            # ---- bias row: cls_b + ws@sfb + wt@tfb  [12, 1] ----
            with tc.high_priority():
                pbias = sp.tile([C, 1], f32, tag="small")
                nc.tensor.matmul(pbias[:], wp(0, 0, 12), wp(0, 84, 85),
                                 start=True, stop=False)
                nc.tensor.matmul(pbias[:], wp(1, 0, 12), wp(1, 84, 85),
                                 start=False, stop=False)
                nc.tensor.matmul(pbias[:], wp(0, 12, 24), wp(0, 85, 86),
                                 start=False, stop=False)
                nc.tensor.matmul(pbias[:], wp(1, 12, 24), wp(1, 85, 86),
                                 start=False, stop=False)
                nc.tensor.matmul(pbias[:], wp(0, 86, 98)[0:1, :],
                                 wp(0, 98, 99)[0:1, :],
                                 start=False, stop=True)
                bias_sb = cp.tile([C, 1], f32, tag="bias_sb")
                nc.scalar.copy(bias_sb[:], pbias[:])

                # ---- DL [48, 12] then block-diagonal [96, 24] fp16 ----
                pd2 = sp.tile([NDEP, C], f32, tag="small")
                nc.tensor.matmul(pd2[:], wp(0, 36, 84), wp(0, 24, 36),
                                 start=True, stop=False)
                pd2_mm = nc.tensor.matmul(pd2[:], wp(1, 36, 84), wp(1, 24, 36),
                                          start=False, stop=True)
                nc.vector.tensor_copy(w_dl[0:48, 0:12], pd2[:])
                nc.vector.tensor_copy(w_dl[64:112, 12:24], pd2[:])

            g_tiles = [None] * NBANK

            ACT_BANKS = (2, 3, 4, 5)  # evicted on ACT; PSUM seeded with t

            def gather_bank(g):
                gt = gp.tile([128, 512], f32, tag="gt")
                g_tiles[g] = gt
                seed = g in ACT_BANKS
                if seed:
                    nc.tensor.matmul(
                        gt[:], wp(0, 99, 227)[0:12, :], midbufT[:],
                        start=True, stop=False, skip_group_check=True,
                    )
                for s in range(4):
                    m = 4 * g + s
                    nc.tensor.matmul(
                        gt[32 * s:32 * s + 32, :], w_dl[0:112, :],
                        oh8[:, 512 * m:512 * (m + 1)],
                        start=not seed, stop=(not seed) or (s == 3),
                        skip_group_check=seed,
                        tile_position=(0, 32 * s),
                    )

            # ---- combined weights w2 (s-phase first: the s_log -> s_all
            # chain gates the epilogue) ----
            w2_t = cp.tile([128, 6 * 2 * C], f16, tag="w2_t")
            pw = sp.tile([128, 6 * 2 * C], f32, tag="small")
            for m in range(6):
                for h0 in range(2):
                    nc.tensor.matmul(
                        pw[:, 12 * m:12 * m + C], sfch(m, h0), wp(h0, 0, 12),
                        start=(h0 == 0), stop=(h0 == 1),
                    )
            nc.vector.tensor_copy(w2_t[:, 0:72], pw[:, 0:72])

            # ---- s projection + s_log ----
            ps = jp.tile([C, RPC], f32, tag="proj")
            for m in range(6):
                nc.tensor.matmul(
                    ps[:], w2_t[:, 12 * m:12 * m + 12], xch(m, 128),
                    start=(m == 0), stop=(m == 5),
                )
            slog_t = cp.tile([C, RPC], f32, tag="slog_t")
            nc.scalar.activation(slog_t[:], ps[:], Identity, bias=bias_sb[:])

            # i_loc(g,s,h,i) = 32s + 16i + 2g + h, so per-(s,i) the 16
            # (g,h) scalars are a contiguous slog_t slice.
            # s_all[32s+12i+c, 2g+h] = slog_t[c, 32s+16i+2g+h]
            sall_engs = [nc.sync, nc.scalar, nc.sync, nc.scalar,
                         nc.sync, nc.scalar, nc.sync, nc.scalar]
            for s in range(4):
                for i in range(2):
                    sall_engs[2 * s + i].dma_start(
                        s_all[32 * s + 12 * i:32 * s + 12 * i + 12, :],
                        slog_t[0:12, 32 * s + 16 * i:32 * s + 16 * i + 16],
                    )

            pw2 = sp.tile([128, 6 * 2 * C], f32, tag="small")
            for m in range(6):
                for h0 in range(2):
                    nc.tensor.matmul(
                        pw2[:, 72 + 12 * m:72 + 12 * m + C], tfch(m, h0),
                        wp(h0, 12, 24),
                        start=(h0 == 0), stop=(h0 == 1),
                    )
            nc.vector.tensor_copy(w2_t[:, 72:144], pw2[:, 72:144])

            pt = jp.tile([C, L], f32, tag="proj")
            for m in range(6):
                nc.tensor.matmul(
                    pt[:], w2_t[:, 72 + 12 * m:72 + 12 * m + 12], xch(m, 256),
                    start=(m == 0), stop=(m == 5),
                )
            midbufT = cp.tile([C, 2 * L], f16, tag="midbufT")
            nc.vector.tensor_copy(midbufT[:, 0:L], pt[:])
            nc.vector.tensor_copy(midbufT[:, L:2 * L], pt[:])

            # unseeded banks can run while midbufT/prepT are still in flight
            gather_bank(0)
            gather_bank(1)

            # ---- t120 [128, 256]: t_log replicated to (s, i, c) rows ----
            prepT = qp.tile([128, L], f32, tag="prepT")
            nc.tensor.matmul(prepT[:], wp(0, 99, 227)[0:12, :],
                             midbufT[:, 0:L], start=True, stop=True)
            t120 = cp.tile([128, L], f32, tag="t120")
            nc.vector.tensor_copy(t120[:], prepT[:])

            # ---- s_all [128, 16]: s_all[32s+12i+c, 2g+h] =
            #      slog_t[c, 16g+4s+2h+i], one strided DMA ----
            # ---- gather banks + epilogue (out = gt + t120 + s) ----
            for g in range(2, NBANK):
                gather_bank(g)
            for g in range(NBANK):
                gt = g_tiles[g]
                ot = op.tile([128, 512], f16, tag="ot")
                for h in range(2):
                    dst = ot[:, 256 * h:256 * (h + 1)]
                    srcp = gt[:, 256 * h:256 * (h + 1)]
                    scol = s_all[:, 2 * g + h:2 * g + h + 1]
                    if g in ACT_BANKS:
                        nc.scalar.activation(dst, srcp, Identity, bias=scol)
                    else:
                        nc.vector.affine_then_add(dst, srcp, t120[:], 1.0,
                                                  scol)
                eng = nc.gpsimd if g < 4 else nc.sync
                eng.dma_start(out_d[128 * g:128 * (g + 1), :], ot[:])

    nc.compile()
    return nc


def _make_consts(s_fc_w, s_fc_b, t_fc_w, t_fc_b, dep_emb, cls_w, cls_b):
    cw = np.asarray(cls_w, np.float32)
    wpack = np.zeros((H, WCOLS), np.float32)
    wpack[:, 0:12] = cw[:, 0:H].T
    wpack[:, 12:24] = cw[:, H:2 * H].T
    wpack[:, 24:36] = cw[:, 2 * H:].T
    wpack[:, 36:84] = np.asarray(dep_emb, np.float32).T
    wpack[:, 84] = np.asarray(s_fc_b, np.float32)
    wpack[:, 85] = np.asarray(t_fc_b, np.float32)
    wpack[0, 86:98] = np.asarray(cls_b, np.float32)
    wpack[0, 98] = 1.0
    # rep128[c, 32s+12i+c] = 1
    for s in range(4):
        for i in range(2):
            for c in range(C):
                wpack[c, 99 + 32 * s + 12 * i + c] = 1.0
    return {"wpk": wpack.astype(np.float16)}


_COLBASE = None


def _marshal_core(n, input_tensor, dg, consts):
    global _COLBASE
    b, half = n // 2, n % 2
    i0 = half * RPC

    xbT = np.roll(input_tensor[b].T, -i0, axis=1)  # [768, 256] rotated j
    sfw = consts["_sfw"]
    tfw = consts["_tfw"]
    big = np.concatenate(
        [sfw[0:128], sfw[128:256]]
        + [xbT[128 * m:128 * (m + 1)] for m in range(6)]
        + [tfw[0:128], tfw[128:256]],
        axis=1,
    ).astype(np.float16)  # [128, 4608]

    dgc = np.roll(np.asarray(dg[b, i0:i0 + RPC]), -i0, axis=1)  # [128, 256]
    if _COLBASE is None:
        # i_loc r = 32s + 16i + 2g + h -> matmul m = 4g + s,
        # oh col = 512m + 256h + j', oh row block = 48i
        r = np.arange(RPC)
        s_, i_, g_, h_ = r // 32, (r % 32) // 16, (r % 16) // 2, r % 2
        _COLBASE = (
            (512 * (4 * g_ + s_) + 256 * h_)[:, None] + np.arange(L)[None, :],
            (64 * i_)[:, None],
        )
    rows = _COLBASE[1] + dgc
    oh = np.zeros((112, 64 * L), np.uint8)
    oh[rows.ravel(), _COLBASE[0].ravel()] = FP8_ONE

    return {"wpk": consts["wpk"], "big": big, "oh": oh}


def _assemble_core(raw, i0):
    # raw [1024, 512] f16: row = 128g + 32s + 12i + c (24:32 pad),
    # col = 256h + j'; i_loc = 32s + 16i + 2g + h,
    # col j' holds global j = (j'+i0) % L
    arr = raw.reshape(8, 4, 32, 2, L)[:, :, :24]    # (g, s, ic, h, j)
    arr = arr.reshape(8, 4, 2, C, 2, L).transpose(1, 2, 0, 4, 3, 5)
    arr = arr.reshape(RPC, C, L)
    return np.roll(arr, i0, axis=2).astype(np.float32)


def kernel(input_tensor, dependency_graph, s_fc_w, s_fc_b, t_fc_w, t_fc_b,
           dep_emb, cls_w, cls_b):
    global _PROGRAM
    from concourse.bass_utils import run_bass_kernel_spmd

    input_tensor = np.asarray(input_tensor, dtype=np.float32)
    dg = np.asarray(dependency_graph)

    consts = _make_consts(s_fc_w, s_fc_b, t_fc_w, t_fc_b, dep_emb,
                          cls_w, cls_b)
    consts["_sfw"] = np.asarray(s_fc_w, np.float32)
    consts["_tfw"] = np.asarray(t_fc_w, np.float32)

    if _PROGRAM is None:
        _PROGRAM = build_program()
    nc = _PROGRAM

    in_maps = [_marshal_core(n, input_tensor, dg, consts) for n in range(NCORES)]
    trace = bool(int(os.environ.get("KERNEL_PROFILE", "0")))
    res = run_bass_kernel_spmd(
        nc, in_maps, core_ids=list(range(NCORES)), trace=trace
    )
    if trace and res.exec_time_ns is not None:
        print(f"HW exec time: {res.exec_time_ns} ns")

    out = np.empty((B, L, C, L), dtype=np.float32)
    for n in range(NCORES):
        b, half = n // 2, n % 2
        i0 = half * RPC
        out[b, i0:i0 + RPC] = _assemble_core(res.results[n]["out"], i0)
    return out
